# revision 10
# baseline (speedup 1.0000x reference)
"""Two-layer GAT (DGL GATConv) on 8 Trainium2 NeuronCores via Bass/Tile.

v2: dst-partitioned graph parallel with dma_gather edge gathers.

  - Nodes are slot-relabeled per core (greedy block assignment balancing
    per-(block, src-window) edge counts); everything on device is in slot
    order, so both layers share one gather-index array and one one-hot
    structure.
  - Per layer: project own slice (h, el, er from one matmul vs an augmented
    weight matrix), pack 512B table rows [h0|1|h1|1|h2|1|h3|1 bf16 | el f32],
    AllGather the table, then gather per-edge src rows with int16 dma_gather
    (4 windows of 2 core-slices each keep indices < 32768), compute edge
    softmax and aggregate per 128-dst block with one bf16 matmul per tile.
    The interleaved "1" columns make the same matmul emit the per-dst softmax
    normalizers. er[dst] is produced on-chip per tile by transposing the
    one-hot on the PE array and multiplying with the SBUF-resident er table.
"""
import sys

sys.path.insert(0, "/opt/trn_rl_repo")

import math
from contextlib import ExitStack

import ml_dtypes
import numpy as np

import concourse.bass as bass
import concourse.mybir as mybir
import concourse.tile as tile
from concourse._compat import with_exitstack
from concourse.masks import make_identity

NEG = 0.2
F = 128
H = 4
OUT = 32
ROWB = 256          # bf16 elems per table row (512 B)
NC = 8
NPC = 12500
NB = 98             # 128-dst blocks per core
ROWS = NB * 128     # 12544 slots per core
SENT = ROWS - 1     # reserved pad slot on every core (block 97 capped at 127)
TROWS = ROWS * NC
WIN = 4
WROWS = 2 * ROWS    # rows per gather window (pair of core slices), < 32768
GROUP = 4           # dst blocks per PSUM accumulation group
SENT_EL = -80.0     # sentinel el -> exp(lrelu(-80+er)) ~ 1e-7

bf16 = mybir.dt.bfloat16
f32 = mybir.dt.float32
i16 = mybir.dt.int16


# ---------------------------------------------------------------- host prep

def _shared_structure(n3):
    """Per-(block, window) tile budgets + global tile ordering."""
    tb = np.full((NB, WIN), 2, np.int64)
    for w in range(WIN):
        big = (np.arange(n3) * NB // n3 + w * 7) % NB
        tb[np.unique(big), w] = 3
    groups = [list(range(g * GROUP, min(NB, (g + 1) * GROUP)))
              for g in range(math.ceil(NB / GROUP))]
    runs = []           # dict(w, t0, tiles=[(b, start, stop)], fin=[blocks])
    tiles_bw = [[None] * WIN for _ in range(NB)]
    t = 0
    for blocks in groups:
        for w in range(WIN):
            tl = []
            for b in blocks:
                tiles_bw[b][w] = np.arange(t + len(tl), t + len(tl) + tb[b, w])
                for k in range(tb[b, w]):
                    tl.append((b, w == 0 and k == 0,
                               w == WIN - 1 and k == tb[b, WIN - 1] - 1))
            runs.append(dict(w=w, t0=t, tiles=tl,
                             fin=blocks if w == WIN - 1 else []))
            t += len(tl)
    return tb, runs, tiles_bw, t


def _assign_blocks(wvec, tb):
    """Greedy: assign dsts (with per-window edge counts) to blocks under
    per-(b,w) capacity tb*128 and per-block dst capacity."""
    cap = tb * 128
    capd = np.full(NB, 128, np.int64)
    capd[NB - 1] = 127          # reserve SENT slot
    deg = wvec.sum(1)
    order = np.argsort(-deg, kind="stable")
    cnt = np.zeros((NB, WIN), np.int64)
    ndst = np.zeros(NB, np.int64)
    blk = np.empty(NPC, np.int64)
    slot_in = np.empty(NPC, np.int64)
    for d in order:
        resid = cap - cnt - wvec[d]
        ok = (resid.min(1) >= 0) & (ndst < capd)
        if not ok.any():
            return None, None
        score = np.where(ok, resid.min(1) * 1000 - ndst, -(10 ** 9))
        b = int(np.argmax(score))
        blk[d] = b
        slot_in[d] = ndst[b]
        cnt[b] += wvec[d]
        ndst[b] += 1
    return blk * 128 + slot_in, cnt


def prep_inputs(src, dst):
    src = np.asarray(src).astype(np.int64)
    dst = np.asarray(dst).astype(np.int64)
    win_edge = src // (2 * NPC)          # gather window of each edge (by src)

    n3 = 8
    while True:
        tb, runs, tiles_bw, T = _shared_structure(n3)
        perms = []
        ecore = []
        ok = True
        for c in range(NC):
            eid = np.nonzero((dst >= c * NPC) & (dst < (c + 1) * NPC))[0]
            d0 = dst[eid] - c * NPC
            wv = win_edge[eid]
            wvec = np.zeros((NPC, WIN), np.int64)
            np.add.at(wvec, (d0, wv), 1)
            perm, _ = _assign_blocks(wvec, tb)
            if perm is None:
                ok = False
                break
            perms.append(perm)
            ecore.append((eid, d0, wv))
        if ok:
            break
        n3 += 4
        assert n3 <= 32, "edge packing infeasible"

    rowof = np.empty(src.max() + 1 if False else NC * NPC, np.int64)
    for c in range(NC):
        rowof[c * NPC:(c + 1) * NPC] = c * ROWS + perms[c]

    per_core = []
    for c in range(NC):
        eid, d0, wv = ecore[c]
        perm = perms[c]
        osflat = np.full(T * 128, SENT, np.int32)
        dlflat = np.full(T * 128, 127, np.float32)
        b_e = perm[d0] // 128
        slot_e = (perm[d0] % 128).astype(np.float32)
        relrow = (rowof[src[eid]] - wv * WROWS).astype(np.int32)
        key = b_e * WIN + wv
        order_e = np.argsort(key, kind="stable")
        counts = np.bincount(key, minlength=NB * WIN)
        off = np.concatenate([[0], np.cumsum(counts)])
        for b in range(NB):
            for w in range(WIN):
                j0, j1 = off[b * WIN + w], off[b * WIN + w + 1]
                if j1 == j0:
                    continue
                es = order_e[j0:j1]
                jj = np.arange(j1 - j0)
                pos = tiles_bw[b][w][jj // 128] * 128 + jj % 128
                osflat[pos] = relrow[es]
                dlflat[pos] = slot_e[es]
        assert osflat.max() < WROWS and osflat.min() >= 0
        osw = np.tile(
            np.ascontiguousarray(
                osflat.astype(np.int16).reshape(T * 8, 16).T), (8, 1))
        dl = np.ascontiguousarray(
            dlflat.reshape(T, 128).T).astype(ml_dtypes.bfloat16)
        per_core.append(dict(osw=osw, dloc=dl))
    return per_core, perms, tb, runs, T


def aug_weights(W, al, ar):
    """[128, 136] f32: [W | W@al_h | W@ar_h]."""
    Wa = np.zeros((F, 136), np.float32)
    Wa[:, :F] = W
    for h in range(H):
        Wa[:, F + h] = W[:, h * OUT:(h + 1) * OUT] @ al[h]
        Wa[:, F + H + h] = W[:, h * OUT:(h + 1) * OUT] @ ar[h]
    return Wa


# ---------------------------------------------------------------- kernel

@with_exitstack
def build_kernel(ctx: ExitStack, tc: tile.TileContext, runs, T, supmax):
    nc = tc.nc

    featT = nc.dram_tensor("featT", [F, ROWS], f32, kind="ExternalInput")
    w1 = nc.dram_tensor("w1aug", [F, 136], f32, kind="ExternalInput")
    w2 = nc.dram_tensor("w2aug", [F, 136], bf16, kind="ExternalInput")
    osw_d = nc.dram_tensor("osw", [128, T * 8], i16, kind="ExternalInput")
    dloc_d = nc.dram_tensor("dloc", [128, T], bf16, kind="ExternalInput")
    iota_d = nc.dram_tensor("iota", [128, 128], bf16, kind="ExternalInput")
    b1_d = nc.dram_tensor("b1rep", [128, F], f32, kind="ExternalInput")
    b2_d = nc.dram_tensor("b2rep", [128, OUT], f32, kind="ExternalInput")
    sent_d = nc.dram_tensor("sentel", [1, 8], bf16, kind="ExternalInput")
    out_ext = nc.dram_tensor("out", [ROWS, OUT], f32, kind="ExternalOutput")

    slice1 = nc.dram_tensor("slice1", [ROWS, ROWB], bf16)
    slice2 = nc.dram_tensor("slice2", [ROWS, ROWB], bf16)
    table1 = nc.dram_tensor("table1", [TROWS, ROWB], bf16, addr_space="Shared")
    table2 = nc.dram_tensor("table2", [TROWS, ROWB], bf16, addr_space="Shared")
    h1r = nc.dram_tensor("h1r", [ROWS, F], bf16)

    core_ids = list(range(NC))

    consts = ctx.enter_context(tc.tile_pool(name="consts", bufs=1))
    offp = ctx.enter_context(tc.tile_pool(name="offp", bufs=1))
    erp = ctx.enter_context(tc.tile_pool(name="erp", bufs=1))

    w1_sb = consts.tile([F, 136], f32)
    w2_sb = consts.tile([F, 136], bf16)
    iota_sb = consts.tile([128, 128], bf16)
    b1_sb = consts.tile([128, F], f32)
    b2_sb = consts.tile([128, OUT], f32)
    ident = consts.tile([128, 128], bf16)
    sent_sb = consts.tile([1, 8], bf16)
    nc.sync.dma_start(w1_sb[:], w1[:])
    nc.sync.dma_start(w2_sb[:], w2[:])
    nc.sync.dma_start(iota_sb[:], iota_d[:])
    nc.sync.dma_start(b1_sb[:], b1_d[:])
    nc.sync.dma_start(b2_sb[:], b2_d[:])
    nc.sync.dma_start(sent_sb[:], sent_d[:])
    make_identity(nc, ident[:])

    osw_sb = offp.tile([128, T * 8], i16)
    dloc_sb = offp.tile([128, T], bf16)
    nc.sync.dma_start(osw_sb[:], osw_d[:])
    nc.sync.dma_start(dloc_sb[:], dloc_d[:])

    er1_sb = erp.tile([128, NB * 4], bf16)
    er2_sb = erp.tile([128, NB * 4], bf16)

    def proj_phase(get_lhsT, w_sb, slice_d, er_sb):
        with tc.tile_pool(name="php", bufs=2, space="PSUM") as php, \
             tc.tile_pool(name="projp", bufs=2) as projp:
            for nt in range(NB):
                ph = php.tile([128, 136], f32, tag="ph")
                nc.tensor.matmul(out=ph[:], lhsT=get_lhsT(nt), rhs=w_sb[:],
                                 start=True, stop=True)
                row_t = projp.tile([128, ROWB], bf16, tag="rowt")
                rv = row_t[:, :].bitcast(f32)
                # plain row: [h(128 bf16) | el f32 (cols 128..135) | zeros]
                nc.scalar.copy(row_t[:, 0:128], ph[:, 0:128])
                nc.vector.memset(row_t[:, 136:ROWB], 0.0)
                nc.vector.tensor_copy(out=rv[:, 64:68], in_=ph[:, 128:132])
                nc.vector.tensor_copy(out=er_sb[:, nt * 4:(nt + 1) * 4],
                                      in_=ph[:, 132:136])
                nc.sync.dma_start(slice_d[nt * 128:(nt + 1) * 128, :], row_t[:])
            tc.strict_bb_all_engine_barrier()
            # sentinel slot: el := -80 (bf16 cols 128:136 = f32 el field)
            nc.sync.dma_start(slice_d[SENT:SENT + 1, 128:136], sent_sb[:])

    def edge_phase(table, er_sb, layer):
        with tc.tile_pool(name="accp", bufs=GROUP, space="PSUM") as accp, \
             tc.tile_pool(name="tpp", bufs=2, space="PSUM") as tpp, \
             tc.tile_pool(name="erps", bufs=2, space="PSUM") as erps, \
             tc.tile_pool(name="gp", bufs=2) as gp, \
             tc.tile_pool(name="ep", bufs=2) as ep, \
             tc.tile_pool(name="outp", bufs=2) as outp:
            acc_t = {}
            for run in runs:
                w, t0, tiles = run["w"], run["t0"], run["tiles"]
                n = len(tiles)
                g_t = gp.tile([128, supmax, ROWB], bf16, tag="g")
                # Q7 dma_gather caps at 1024 indices per call
                for c0 in range(0, n, 8):
                    cn = min(8, n - c0)
                    nc.gpsimd.dma_gather(
                        g_t[:, c0:c0 + cn, :],
                        table[w * WROWS:(w + 1) * WROWS, :],
                        osw_sb[:, (t0 + c0) * 8:(t0 + c0 + cn) * 8],
                        cn * 128, cn * 128, ROWB)

                s0_t = ep.tile([128, supmax * 128], bf16, tag="s0")
                io = iota_sb[:, :]
                dl = dloc_sb[:, t0:t0 + n]
                iota_ap = bass.AP(tensor=io.tensor, offset=io.offset,
                                  ap=[io.ap[0], [0, n], [1, 128]])
                dloc_ap = bass.AP(tensor=dl.tensor, offset=dl.offset,
                                  ap=[dl.ap[0], [1, n], [0, 128]])
                nc.vector.tensor_tensor(out=s0_t[:, :n * 128], in0=iota_ap,
                                        in1=dloc_ap,
                                        op=mybir.AluOpType.is_equal)

                er_ps = erps.tile([128, supmax * 4], f32, tag="erps")
                for k, (b, st, sp) in enumerate(tiles):
                    s0T_ps = tpp.tile([128, 128], bf16, tag="s0T")
                    nc.tensor.transpose(s0T_ps[:],
                                        s0_t[:, k * 128:(k + 1) * 128],
                                        ident[:])
                    s0T_sb = ep.tile([128, 128], bf16, tag="s0Ts")
                    if k % 2 == 0:
                        nc.scalar.copy(s0T_sb[:], s0T_ps[:])
                    else:
                        nc.vector.tensor_copy(s0T_sb[:], s0T_ps[:])
                    nc.tensor.matmul(out=er_ps[:, k * 4:(k + 1) * 4],
                                     lhsT=s0T_sb[:],
                                     rhs=er_sb[:, b * 4:(b + 1) * 4],
                                     start=True, stop=True,
                                     skip_group_check=True)

                g32 = g_t[:, :, :].bitcast(f32)
                el_ap = bass.AP(tensor=g32.tensor, offset=g32.offset + 64,
                                ap=[g32.ap[0], [128, n], [1, 4]])
                lg = ep.tile([128, supmax * 4], f32, tag="lg")
                nc.vector.tensor_tensor(out=lg[:, :n * 4], in0=el_ap,
                                        in1=er_ps[:, :n * 4],
                                        op=mybir.AluOpType.add)
                lrl = ep.tile([128, supmax * 4], f32, tag="lrl")
                nc.vector.tensor_scalar_mul(lrl[:, :n * 4], lg[:, :n * 4], NEG)
                nc.vector.tensor_tensor(out=lrl[:, :n * 4], in0=lg[:, :n * 4],
                                        in1=lrl[:, :n * 4],
                                        op=mybir.AluOpType.max)
                p_t = ep.tile([128, supmax * 4], bf16, tag="p")
                nc.scalar.activation(p_t[:, :n * 4], lrl[:, :n * 4],
                                     mybir.ActivationFunctionType.Exp)

                rhs_t = ep.tile([128, supmax * 132], bf16, tag="rhs")
                gb, pb, rb = g_t[:, :, :], p_t[:, :], rhs_t[:, :]
                for hd in range(H):
                    in0 = bass.AP(tensor=gb.tensor, offset=gb.offset + hd * 32,
                                  ap=[gb.ap[0], [ROWB, n], [1, 32]])
                    in1 = bass.AP(tensor=pb.tensor, offset=pb.offset + hd,
                                  ap=[pb.ap[0], [4, n], [0, 32]])
                    o = bass.AP(tensor=rb.tensor, offset=rb.offset + hd * 32,
                                ap=[rb.ap[0], [132, n], [1, 32]])
                    nc.vector.tensor_tensor(out=o, in0=in0, in1=in1,
                                            op=mybir.AluOpType.mult)
                pco = bass.AP(tensor=rb.tensor, offset=rb.offset + 128,
                              ap=[rb.ap[0], [132, n], [1, 4]])
                pci = bass.AP(tensor=pb.tensor, offset=pb.offset,
                              ap=[pb.ap[0], [4, n], [1, 4]])
                nc.vector.tensor_copy(out=pco, in_=pci)

                for k, (b, st, sp) in enumerate(tiles):
                    if st:
                        acc_t[b] = accp.tile([128, 132], f32, tag="acc",
                                             name=f"acc_l{layer}_b{b}")
                    nc.tensor.matmul(out=acc_t[b][:],
                                     lhsT=s0_t[:, k * 128:(k + 1) * 128],
                                     rhs=rhs_t[:, k * 132:(k + 1) * 132],
                                     start=st, stop=sp, skip_group_check=True)

                if run["fin"]:
                    finalize(run["fin"], acc_t, layer, outp)

    def finalize(blocks, acc_t, layer, outp):
        nb = len(blocks)
        stag = outp.tile([128, GROUP * 132], f32, tag="stag")
        for j, b in enumerate(blocks):
            nc.scalar.copy(stag[:, j * 132:(j + 1) * 132], acc_t[b][:])
        sv = stag[:, :]
        s_ap = bass.AP(tensor=sv.tensor, offset=sv.offset + 128,
                       ap=[sv.ap[0], [132, nb], [1, 4]])
        seps = outp.tile([128, GROUP * 4], f32, tag="seps")
        nc.vector.tensor_scalar_add(seps[:, :nb * 4], s_ap, 1e-30)
        inv = outp.tile([128, GROUP * 4], f32, tag="inv")
        nc.vector.reciprocal(inv[:, :nb * 4], seps[:, :nb * 4])
        if layer == 2:
            nc.vector.tensor_scalar_mul(inv[:, :nb * 4], inv[:, :nb * 4], 0.25)
        of = outp.tile([128, GROUP * 128], f32, tag="of")
        ov, iv = of[:, :], inv[:, :]
        for hd in range(H):
            msg_ap = bass.AP(tensor=sv.tensor, offset=sv.offset + hd * 32,
                             ap=[sv.ap[0], [132, nb], [1, 32]])
            inv_ap = bass.AP(tensor=iv.tensor, offset=iv.offset + hd,
                             ap=[iv.ap[0], [4, nb], [0, 32]])
            of_ap = bass.AP(tensor=ov.tensor, offset=ov.offset + hd * 32,
                            ap=[ov.ap[0], [128, nb], [1, 32]])
            nc.vector.tensor_tensor(out=of_ap, in0=msg_ap, in1=inv_ap,
                                    op=mybir.AluOpType.mult)
        if layer == 1:
            bv = b1_sb[:, :]
            b1_ap = bass.AP(tensor=bv.tensor, offset=bv.offset,
                            ap=[bv.ap[0], [0, nb], [1, F]])
            of2_ap = bass.AP(tensor=ov.tensor, offset=ov.offset,
                             ap=[ov.ap[0], [128, nb], [1, F]])
            nc.vector.tensor_tensor(out=of2_ap, in0=of2_ap, in1=b1_ap,
                                    op=mybir.AluOpType.add)
            h1row = outp.tile([128, GROUP * 128], bf16, tag="h1row")
            nc.scalar.activation(h1row[:, :nb * 128], of[:, :nb * 128],
                                 mybir.ActivationFunctionType.Relu)
            for j, b in enumerate(blocks):
                nc.sync.dma_start(h1r[b * 128:(b + 1) * 128, :],
                                  h1row[:, j * 128:(j + 1) * 128])
        else:
            # mean over heads = ((h0+h1) + (h2+h3)) with 0.25 folded into inv
            rd = outp.tile([128, GROUP * OUT], f32, tag="rd")
            rd2 = outp.tile([128, GROUP * OUT], f32, tag="rd2")
            aps = [bass.AP(tensor=ov.tensor, offset=ov.offset + hd * 32,
                           ap=[ov.ap[0], [128, nb], [1, 32]]) for hd in range(H)]
            rda = bass.AP(tensor=rd[:, :].tensor, offset=rd[:, :].offset,
                          ap=[rd[:, :].ap[0], [32, nb], [1, 32]])
            rda2 = bass.AP(tensor=rd2[:, :].tensor, offset=rd2[:, :].offset,
                           ap=[rd2[:, :].ap[0], [32, nb], [1, 32]])
            nc.vector.tensor_tensor(out=rda, in0=aps[0], in1=aps[1],
                                    op=mybir.AluOpType.add)
            nc.vector.tensor_tensor(out=rda2, in0=aps[2], in1=aps[3],
                                    op=mybir.AluOpType.add)
            nc.vector.tensor_tensor(out=rda, in0=rda, in1=rda2,
                                    op=mybir.AluOpType.add)
            bv2 = b2_sb[:, :]
            b2_ap = bass.AP(tensor=bv2.tensor, offset=bv2.offset,
                            ap=[bv2.ap[0], [0, nb], [1, OUT]])
            nc.vector.tensor_tensor(out=rda, in0=rda, in1=b2_ap,
                                    op=mybir.AluOpType.add)
            for j, b in enumerate(blocks):
                nc.sync.dma_start(out_ext[b * 128:(b + 1) * 128, :],
                                  rd[:, j * OUT:(j + 1) * OUT])

    # ---------------- phase sequence ----------------
    with tc.tile_pool(name="featp", bufs=1) as featp:
        featT_sb = featp.tile([F, ROWS], f32)
        nc.sync.dma_start(featT_sb[:], featT[:])
        proj_phase(lambda nt: featT_sb[:, nt * 128:(nt + 1) * 128],
                   w1_sb, slice1, er1_sb)
        tc.strict_bb_all_engine_barrier()
        nc.gpsimd.collective_compute(
            "AllGather", mybir.AluOpType.bypass, replica_groups=[core_ids],
            ins=[slice1[:]], outs=[table1[:]])
        tc.strict_bb_all_engine_barrier()
        edge_phase(table1, er1_sb, 1)

    tc.strict_bb_all_engine_barrier()
    with tc.tile_pool(name="h1p", bufs=1) as h1p:
        h1T_sb = h1p.tile([128, ROWS], bf16)
        nc.sync.dma_start(h1T_sb[:], h1r[:], transpose=True)
        proj_phase(lambda nt: h1T_sb[:, nt * 128:(nt + 1) * 128],
                   w2_sb, slice2, er2_sb)
        tc.strict_bb_all_engine_barrier()
        nc.gpsimd.collective_compute(
            "AllGather", mybir.AluOpType.bypass, replica_groups=[core_ids],
            ins=[slice2[:]], outs=[table2[:]])
        tc.strict_bb_all_engine_barrier()
        edge_phase(table2, er2_sb, 2)


def build_nc(runs, T, compile=True):
    from concourse import bacc

    supmax = max(len(r["tiles"]) for r in runs)
    nc = bacc.Bacc("TRN2", target_bir_lowering=False)
    with tile.TileContext(nc) as tc:
        build_kernel(tc, runs, T, supmax)
    if compile:
        nc.compile()
    return nc


def make_in_maps(per_core, feat, perms, W1, al1, ar1, b1, W2, al2, ar2, b2):
    w1a = aug_weights(np.asarray(W1, np.float32), np.asarray(al1, np.float32),
                      np.asarray(ar1, np.float32))
    w2a = aug_weights(np.asarray(W2, np.float32), np.asarray(al2, np.float32),
                      np.asarray(ar2, np.float32)).astype(ml_dtypes.bfloat16)
    iota = np.broadcast_to(np.arange(128, dtype=np.float32), (128, 128))
    iota = np.ascontiguousarray(iota.astype(ml_dtypes.bfloat16))
    sentel = np.full((1, 4), SENT_EL, np.float32).view(np.uint16).reshape(1, 8)
    sentel = sentel.view(ml_dtypes.bfloat16)
    b1r = np.ascontiguousarray(np.broadcast_to(
        np.asarray(b1, np.float32).reshape(1, F), (128, F)))
    b2m = np.asarray(b2, np.float32).reshape(H, OUT).mean(axis=0)
    b2r = np.ascontiguousarray(np.broadcast_to(b2m.reshape(1, OUT), (128, OUT)))
    feat = np.asarray(feat, np.float32)
    in_maps = []
    for c in range(NC):
        fs = np.zeros((ROWS, F), np.float32)
        fs[perms[c]] = feat[c * NPC:(c + 1) * NPC]
        m = dict(
            featT=np.ascontiguousarray(fs.T),
            w1aug=w1a, w2aug=w2a,
            osw=per_core[c]["osw"],
            dloc=per_core[c]["dloc"],
            iota=iota, b1rep=b1r, b2rep=b2r, sentel=sentel,
        )
        in_maps.append(m)
    return in_maps


_CACHE = {}


def _get_program(src, dst):
    per_core, perms, tb, runs, T = prep_inputs(src, dst)
    key = (T, tb.tobytes())
    if key not in _CACHE:
        _CACHE[key] = build_nc(runs, T)
    return _CACHE[key], per_core, perms


def kernel(feat, src, dst, W1, al1, ar1, b1, W2, al2, ar2, b2,
           _trace=False, _return_results=False):
    from concourse.bass_utils import run_bass_kernel_spmd

    nc, per_core, perms = _get_program(src, dst)
    in_maps = make_in_maps(per_core, feat, perms, W1, al1, ar1, b1,
                           W2, al2, ar2, b2)
    res = run_bass_kernel_spmd(nc, in_maps, list(range(NC)), trace=_trace)
    out = np.zeros((NC * NPC, OUT), np.float32)
    for c in range(NC):
        oc = np.asarray(res.results[c]["out"])
        out[c * NPC:(c + 1) * NPC] = oc[perms[c]]
    if _return_results:
        return out, res
    return out


# revision 11
# speedup vs baseline: 1.3005x; 1.3005x over previous
"""Two-layer GAT (DGL GATConv) on 8 Trainium2 NeuronCores via Bass/Tile.

v2: dst-partitioned graph parallel with dma_gather edge gathers.

  - Nodes are slot-relabeled per core (greedy block assignment balancing
    per-(block, src-window) edge counts); everything on device is in slot
    order, so both layers share one gather-index array and one one-hot
    structure.
  - Per layer: project own slice (h, el, er from one matmul vs an augmented
    weight matrix), pack 512B table rows [h0|1|h1|1|h2|1|h3|1 bf16 | el f32],
    AllGather the table, then gather per-edge src rows with int16 dma_gather
    (4 windows of 2 core-slices each keep indices < 32768), compute edge
    softmax and aggregate per 128-dst block with one bf16 matmul per tile.
    The interleaved "1" columns make the same matmul emit the per-dst softmax
    normalizers. er[dst] is produced on-chip per tile by transposing the
    one-hot on the PE array and multiplying with the SBUF-resident er table.
"""
import sys

sys.path.insert(0, "/opt/trn_rl_repo")

import math
from contextlib import ExitStack

import ml_dtypes
import numpy as np

import concourse.bass as bass
import concourse.mybir as mybir
import concourse.tile as tile
from concourse._compat import with_exitstack
from concourse.masks import make_identity

NEG = 0.2
F = 128
H = 4
OUT = 32
ROWB = 256          # bf16 elems per table row (512 B)
NC = 8
NPC = 12500
NB = 98             # 128-dst blocks per core
ROWS = NB * 128     # 12544 slots per core
SENT = ROWS - 1     # reserved pad slot on every core (block 97 capped at 127)
TROWS = ROWS * NC
WIN = 4
WROWS = 2 * ROWS    # rows per gather window (pair of core slices), < 32768
GROUP = 4           # dst blocks per PSUM accumulation group
SENT_EL = -80.0     # sentinel el -> exp(lrelu(-80+er)) ~ 1e-7

bf16 = mybir.dt.bfloat16
f32 = mybir.dt.float32
i16 = mybir.dt.int16


# ---------------------------------------------------------------- host prep

def _shared_structure(n3):
    """Per-(block, window) tile budgets + global tile ordering."""
    tb = np.full((NB, WIN), 2, np.int64)
    for w in range(WIN):
        big = (np.arange(n3) * NB // n3 + w * 7) % NB
        tb[np.unique(big), w] = 3
    groups = [list(range(g * GROUP, min(NB, (g + 1) * GROUP)))
              for g in range(math.ceil(NB / GROUP))]
    runs = []           # dict(w, t0, tiles=[(b, start, stop)], fin=[blocks])
    tiles_bw = [[None] * WIN for _ in range(NB)]
    t = 0
    for blocks in groups:
        for w in range(WIN):
            tl = []
            for b in blocks:
                tiles_bw[b][w] = np.arange(t + len(tl), t + len(tl) + tb[b, w])
                for k in range(tb[b, w]):
                    tl.append((b, w == 0 and k == 0,
                               w == WIN - 1 and k == tb[b, WIN - 1] - 1))
            runs.append(dict(w=w, t0=t, tiles=tl,
                             fin=blocks if w == WIN - 1 else []))
            t += len(tl)
    return tb, runs, tiles_bw, t


def _assign_blocks(wvec, tb):
    """Greedy: assign dsts (with per-window edge counts) to blocks under
    per-(b,w) capacity tb*128 and per-block dst capacity."""
    cap = tb * 128
    capd = np.full(NB, 128, np.int64)
    capd[NB - 1] = 127          # reserve SENT slot
    deg = wvec.sum(1)
    order = np.argsort(-deg, kind="stable")
    cnt = np.zeros((NB, WIN), np.int64)
    ndst = np.zeros(NB, np.int64)
    blk = np.empty(NPC, np.int64)
    slot_in = np.empty(NPC, np.int64)
    for d in order:
        resid = cap - cnt - wvec[d]
        ok = (resid.min(1) >= 0) & (ndst < capd)
        if not ok.any():
            return None, None
        score = np.where(ok, resid.min(1) * 1000 - ndst, -(10 ** 9))
        b = int(np.argmax(score))
        blk[d] = b
        slot_in[d] = ndst[b]
        cnt[b] += wvec[d]
        ndst[b] += 1
    return blk * 128 + slot_in, cnt


def prep_inputs(src, dst):
    src = np.asarray(src).astype(np.int64)
    dst = np.asarray(dst).astype(np.int64)
    win_edge = src // (2 * NPC)          # gather window of each edge (by src)

    n3 = 8
    while True:
        tb, runs, tiles_bw, T = _shared_structure(n3)
        perms = []
        ecore = []
        ok = True
        for c in range(NC):
            eid = np.nonzero((dst >= c * NPC) & (dst < (c + 1) * NPC))[0]
            d0 = dst[eid] - c * NPC
            wv = win_edge[eid]
            wvec = np.zeros((NPC, WIN), np.int64)
            np.add.at(wvec, (d0, wv), 1)
            perm, _ = _assign_blocks(wvec, tb)
            if perm is None:
                ok = False
                break
            perms.append(perm)
            ecore.append((eid, d0, wv))
        if ok:
            break
        n3 += 4
        assert n3 <= 32, "edge packing infeasible"

    rowof = np.empty(src.max() + 1 if False else NC * NPC, np.int64)
    for c in range(NC):
        rowof[c * NPC:(c + 1) * NPC] = c * ROWS + perms[c]

    per_core = []
    for c in range(NC):
        eid, d0, wv = ecore[c]
        perm = perms[c]
        osflat = np.full(T * 128, SENT, np.int32)
        dlflat = np.full(T * 128, 127, np.float32)
        b_e = perm[d0] // 128
        slot_e = (perm[d0] % 128).astype(np.float32)
        relrow = (rowof[src[eid]] - wv * WROWS).astype(np.int32)
        key = b_e * WIN + wv
        order_e = np.argsort(key, kind="stable")
        counts = np.bincount(key, minlength=NB * WIN)
        off = np.concatenate([[0], np.cumsum(counts)])
        for b in range(NB):
            for w in range(WIN):
                j0, j1 = off[b * WIN + w], off[b * WIN + w + 1]
                if j1 == j0:
                    continue
                es = order_e[j0:j1]
                jj = np.arange(j1 - j0)
                pos = tiles_bw[b][w][jj // 128] * 128 + jj % 128
                osflat[pos] = relrow[es]
                dlflat[pos] = slot_e[es]
        assert osflat.max() < WROWS and osflat.min() >= 0
        osw = np.tile(
            np.ascontiguousarray(
                osflat.astype(np.int16).reshape(T * 8, 16).T), (8, 1))
        dl = np.ascontiguousarray(
            dlflat.reshape(T, 128).T).astype(ml_dtypes.bfloat16)
        per_core.append(dict(osw=osw, dloc=dl))
    return per_core, perms, tb, runs, T


def aug_weights(W, al, ar):
    """[128, 136] f32: [W | W@al_h | W@ar_h]."""
    Wa = np.zeros((F, 136), np.float32)
    Wa[:, :F] = W
    for h in range(H):
        Wa[:, F + h] = W[:, h * OUT:(h + 1) * OUT] @ al[h]
        Wa[:, F + H + h] = W[:, h * OUT:(h + 1) * OUT] @ ar[h]
    return Wa


# ---------------------------------------------------------------- kernel

@with_exitstack
def build_kernel(ctx: ExitStack, tc: tile.TileContext, runs, T, supmax):
    nc = tc.nc

    featT = nc.dram_tensor("featT", [F, ROWS], f32, kind="ExternalInput")
    w1 = nc.dram_tensor("w1aug", [F, 136], f32, kind="ExternalInput")
    w2 = nc.dram_tensor("w2aug", [F, 136], bf16, kind="ExternalInput")
    osw_d = nc.dram_tensor("osw", [128, T * 8], i16, kind="ExternalInput")
    dloc_d = nc.dram_tensor("dloc", [128, T], bf16, kind="ExternalInput")
    iota_d = nc.dram_tensor("iota", [128, 128], bf16, kind="ExternalInput")
    b1_d = nc.dram_tensor("b1rep", [128, F], f32, kind="ExternalInput")
    b2_d = nc.dram_tensor("b2rep", [128, OUT], f32, kind="ExternalInput")
    sent_d = nc.dram_tensor("sentel", [1, 8], bf16, kind="ExternalInput")
    out_ext = nc.dram_tensor("out", [ROWS, OUT], f32, kind="ExternalOutput")

    slice1 = nc.dram_tensor("slice1", [ROWS, ROWB], bf16)
    slice2 = nc.dram_tensor("slice2", [ROWS, ROWB], bf16)
    table1 = nc.dram_tensor("table1", [TROWS, ROWB], bf16, addr_space="Shared")
    table2 = nc.dram_tensor("table2", [TROWS, ROWB], bf16, addr_space="Shared")
    h1r = nc.dram_tensor("h1r", [ROWS, F], bf16)

    core_ids = list(range(NC))

    consts = ctx.enter_context(tc.tile_pool(name="consts", bufs=1))
    offp = ctx.enter_context(tc.tile_pool(name="offp", bufs=1))
    erp = ctx.enter_context(tc.tile_pool(name="erp", bufs=1))

    w1_sb = consts.tile([F, 136], f32)
    w2_sb = consts.tile([F, 136], bf16)
    iota_sb = consts.tile([128, 128], bf16)
    b1_sb = consts.tile([128, F], f32)
    b2_sb = consts.tile([128, OUT], f32)
    ident = consts.tile([128, 128], bf16)
    sent_sb = consts.tile([1, 8], bf16)
    nc.sync.dma_start(w1_sb[:], w1[:])
    nc.sync.dma_start(w2_sb[:], w2[:])
    nc.sync.dma_start(iota_sb[:], iota_d[:])
    nc.sync.dma_start(b1_sb[:], b1_d[:])
    nc.sync.dma_start(b2_sb[:], b2_d[:])
    nc.sync.dma_start(sent_sb[:], sent_d[:])
    make_identity(nc, ident[:])

    osw_sb = offp.tile([128, T * 8], i16)
    dloc_sb = offp.tile([128, T], bf16)
    nc.sync.dma_start(osw_sb[:], osw_d[:])
    nc.sync.dma_start(dloc_sb[:], dloc_d[:])

    er1_sb = erp.tile([128, NB * 4], bf16)
    er2_sb = erp.tile([128, NB * 4], bf16)

    def proj_phase(get_lhsT, w_sb, slice_d, er_sb):
        with tc.tile_pool(name="php", bufs=2, space="PSUM") as php, \
             tc.tile_pool(name="projp", bufs=2) as projp:
            for nt in range(NB):
                ph = php.tile([128, 136], f32, tag="ph")
                nc.tensor.matmul(out=ph[:], lhsT=get_lhsT(nt), rhs=w_sb[:],
                                 start=True, stop=True)
                row_t = projp.tile([128, ROWB], bf16, tag="rowt")
                rv = row_t[:, :].bitcast(f32)
                # plain row: [h(128 bf16) | el f32 (cols 128..135) | zeros]
                nc.scalar.copy(row_t[:, 0:128], ph[:, 0:128])
                nc.vector.memset(row_t[:, 136:ROWB], 0.0)
                nc.vector.tensor_copy(out=rv[:, 64:68], in_=ph[:, 128:132])
                nc.vector.tensor_copy(out=er_sb[:, nt * 4:(nt + 1) * 4],
                                      in_=ph[:, 132:136])
                nc.sync.dma_start(slice_d[nt * 128:(nt + 1) * 128, :], row_t[:])
            tc.strict_bb_all_engine_barrier()
            # sentinel slot: el := -80 (bf16 cols 128:136 = f32 el field)
            nc.sync.dma_start(slice_d[SENT:SENT + 1, 128:136], sent_sb[:])

    def edge_phase(table, er_sb, layer):
        with tc.tile_pool(name="accp", bufs=GROUP, space="PSUM") as accp, \
             tc.tile_pool(name="tpp", bufs=2, space="PSUM") as tpp, \
             tc.tile_pool(name="erps", bufs=2, space="PSUM") as erps, \
             tc.tile_pool(name="gp", bufs=2) as gp, \
             tc.tile_pool(name="ep", bufs=2) as ep, \
             tc.tile_pool(name="outp", bufs=2) as outp:
            acc_t = {}
            qn = [0]
            for run in runs:
                w, t0, tiles = run["w"], run["t0"], run["tiles"]
                n = len(tiles)
                g_t = gp.tile([128, supmax, ROWB], bf16, tag="g")
                # Q7 dma_gather caps at 1024 indices per call; spread
                # calls over the 4 SWDGE queues so SDMA engines interleave
                # packets (more outstanding HBM reads).
                for c0 in range(0, n, 4):
                    cn = min(4, n - c0)
                    nc.gpsimd.dma_gather(
                        g_t[:, c0:c0 + cn, :],
                        table[w * WROWS:(w + 1) * WROWS, :],
                        osw_sb[:, (t0 + c0) * 8:(t0 + c0 + cn) * 8],
                        cn * 128, cn * 128, ROWB,
                        single_packet=False,
                        queue_num=(qn[0] % 4))
                    qn[0] += 1

                s0_t = ep.tile([128, supmax * 128], bf16, tag="s0")
                io = iota_sb[:, :]
                dl = dloc_sb[:, t0:t0 + n]
                iota_ap = bass.AP(tensor=io.tensor, offset=io.offset,
                                  ap=[io.ap[0], [0, n], [1, 128]])
                dloc_ap = bass.AP(tensor=dl.tensor, offset=dl.offset,
                                  ap=[dl.ap[0], [1, n], [0, 128]])
                nc.vector.tensor_tensor(out=s0_t[:, :n * 128], in0=iota_ap,
                                        in1=dloc_ap,
                                        op=mybir.AluOpType.is_equal)

                er_ps = erps.tile([128, supmax * 4], f32, tag="erps")
                for k, (b, st, sp) in enumerate(tiles):
                    s0T_ps = tpp.tile([128, 128], bf16, tag="s0T")
                    nc.tensor.transpose(s0T_ps[:],
                                        s0_t[:, k * 128:(k + 1) * 128],
                                        ident[:])
                    s0T_sb = ep.tile([128, 128], bf16, tag="s0Ts")
                    if k % 2 == 0:
                        nc.scalar.copy(s0T_sb[:], s0T_ps[:])
                    else:
                        nc.vector.tensor_copy(s0T_sb[:], s0T_ps[:])
                    nc.tensor.matmul(out=er_ps[:, k * 4:(k + 1) * 4],
                                     lhsT=s0T_sb[:],
                                     rhs=er_sb[:, b * 4:(b + 1) * 4],
                                     start=True, stop=True,
                                     skip_group_check=True)

                g32 = g_t[:, :, :].bitcast(f32)
                el_ap = bass.AP(tensor=g32.tensor, offset=g32.offset + 64,
                                ap=[g32.ap[0], [128, n], [1, 4]])
                lg = ep.tile([128, supmax * 4], f32, tag="lg")
                nc.vector.tensor_tensor(out=lg[:, :n * 4], in0=el_ap,
                                        in1=er_ps[:, :n * 4],
                                        op=mybir.AluOpType.add)
                lrl = ep.tile([128, supmax * 4], f32, tag="lrl")
                nc.vector.tensor_scalar_mul(lrl[:, :n * 4], lg[:, :n * 4], NEG)
                nc.vector.tensor_tensor(out=lrl[:, :n * 4], in0=lg[:, :n * 4],
                                        in1=lrl[:, :n * 4],
                                        op=mybir.AluOpType.max)
                p_t = ep.tile([128, supmax * 4], bf16, tag="p")
                nc.scalar.activation(p_t[:, :n * 4], lrl[:, :n * 4],
                                     mybir.ActivationFunctionType.Exp)

                rhs_t = ep.tile([128, supmax * 132], bf16, tag="rhs")
                gb, pb, rb = g_t[:, :, :], p_t[:, :], rhs_t[:, :]
                for hd in range(H):
                    in0 = bass.AP(tensor=gb.tensor, offset=gb.offset + hd * 32,
                                  ap=[gb.ap[0], [ROWB, n], [1, 32]])
                    in1 = bass.AP(tensor=pb.tensor, offset=pb.offset + hd,
                                  ap=[pb.ap[0], [4, n], [0, 32]])
                    o = bass.AP(tensor=rb.tensor, offset=rb.offset + hd * 32,
                                ap=[rb.ap[0], [132, n], [1, 32]])
                    nc.vector.tensor_tensor(out=o, in0=in0, in1=in1,
                                            op=mybir.AluOpType.mult)
                pco = bass.AP(tensor=rb.tensor, offset=rb.offset + 128,
                              ap=[rb.ap[0], [132, n], [1, 4]])
                pci = bass.AP(tensor=pb.tensor, offset=pb.offset,
                              ap=[pb.ap[0], [4, n], [1, 4]])
                nc.vector.tensor_copy(out=pco, in_=pci)

                for k, (b, st, sp) in enumerate(tiles):
                    if st:
                        acc_t[b] = accp.tile([128, 132], f32, tag="acc",
                                             name=f"acc_l{layer}_b{b}")
                    nc.tensor.matmul(out=acc_t[b][:],
                                     lhsT=s0_t[:, k * 128:(k + 1) * 128],
                                     rhs=rhs_t[:, k * 132:(k + 1) * 132],
                                     start=st, stop=sp, skip_group_check=True)

                if run["fin"]:
                    finalize(run["fin"], acc_t, layer, outp)

    def finalize(blocks, acc_t, layer, outp):
        nb = len(blocks)
        stag = outp.tile([128, GROUP * 132], f32, tag="stag")
        for j, b in enumerate(blocks):
            nc.scalar.copy(stag[:, j * 132:(j + 1) * 132], acc_t[b][:])
        sv = stag[:, :]
        s_ap = bass.AP(tensor=sv.tensor, offset=sv.offset + 128,
                       ap=[sv.ap[0], [132, nb], [1, 4]])
        seps = outp.tile([128, GROUP * 4], f32, tag="seps")
        nc.vector.tensor_scalar_add(seps[:, :nb * 4], s_ap, 1e-30)
        inv = outp.tile([128, GROUP * 4], f32, tag="inv")
        nc.vector.reciprocal(inv[:, :nb * 4], seps[:, :nb * 4])
        if layer == 2:
            nc.vector.tensor_scalar_mul(inv[:, :nb * 4], inv[:, :nb * 4], 0.25)
        of = outp.tile([128, GROUP * 128], f32, tag="of")
        ov, iv = of[:, :], inv[:, :]
        for hd in range(H):
            msg_ap = bass.AP(tensor=sv.tensor, offset=sv.offset + hd * 32,
                             ap=[sv.ap[0], [132, nb], [1, 32]])
            inv_ap = bass.AP(tensor=iv.tensor, offset=iv.offset + hd,
                             ap=[iv.ap[0], [4, nb], [0, 32]])
            of_ap = bass.AP(tensor=ov.tensor, offset=ov.offset + hd * 32,
                            ap=[ov.ap[0], [128, nb], [1, 32]])
            nc.vector.tensor_tensor(out=of_ap, in0=msg_ap, in1=inv_ap,
                                    op=mybir.AluOpType.mult)
        if layer == 1:
            bv = b1_sb[:, :]
            b1_ap = bass.AP(tensor=bv.tensor, offset=bv.offset,
                            ap=[bv.ap[0], [0, nb], [1, F]])
            of2_ap = bass.AP(tensor=ov.tensor, offset=ov.offset,
                             ap=[ov.ap[0], [128, nb], [1, F]])
            nc.vector.tensor_tensor(out=of2_ap, in0=of2_ap, in1=b1_ap,
                                    op=mybir.AluOpType.add)
            h1row = outp.tile([128, GROUP * 128], bf16, tag="h1row")
            nc.scalar.activation(h1row[:, :nb * 128], of[:, :nb * 128],
                                 mybir.ActivationFunctionType.Relu)
            for j, b in enumerate(blocks):
                nc.sync.dma_start(h1r[b * 128:(b + 1) * 128, :],
                                  h1row[:, j * 128:(j + 1) * 128])
        else:
            # mean over heads = ((h0+h1) + (h2+h3)) with 0.25 folded into inv
            rd = outp.tile([128, GROUP * OUT], f32, tag="rd")
            rd2 = outp.tile([128, GROUP * OUT], f32, tag="rd2")
            aps = [bass.AP(tensor=ov.tensor, offset=ov.offset + hd * 32,
                           ap=[ov.ap[0], [128, nb], [1, 32]]) for hd in range(H)]
            rda = bass.AP(tensor=rd[:, :].tensor, offset=rd[:, :].offset,
                          ap=[rd[:, :].ap[0], [32, nb], [1, 32]])
            rda2 = bass.AP(tensor=rd2[:, :].tensor, offset=rd2[:, :].offset,
                           ap=[rd2[:, :].ap[0], [32, nb], [1, 32]])
            nc.vector.tensor_tensor(out=rda, in0=aps[0], in1=aps[1],
                                    op=mybir.AluOpType.add)
            nc.vector.tensor_tensor(out=rda2, in0=aps[2], in1=aps[3],
                                    op=mybir.AluOpType.add)
            nc.vector.tensor_tensor(out=rda, in0=rda, in1=rda2,
                                    op=mybir.AluOpType.add)
            bv2 = b2_sb[:, :]
            b2_ap = bass.AP(tensor=bv2.tensor, offset=bv2.offset,
                            ap=[bv2.ap[0], [0, nb], [1, OUT]])
            nc.vector.tensor_tensor(out=rda, in0=rda, in1=b2_ap,
                                    op=mybir.AluOpType.add)
            for j, b in enumerate(blocks):
                nc.sync.dma_start(out_ext[b * 128:(b + 1) * 128, :],
                                  rd[:, j * OUT:(j + 1) * OUT])

    # ---------------- phase sequence ----------------
    with tc.tile_pool(name="featp", bufs=1) as featp:
        featT_sb = featp.tile([F, ROWS], f32)
        nc.sync.dma_start(featT_sb[:], featT[:])
        proj_phase(lambda nt: featT_sb[:, nt * 128:(nt + 1) * 128],
                   w1_sb, slice1, er1_sb)
        tc.strict_bb_all_engine_barrier()
        nc.gpsimd.collective_compute(
            "AllGather", mybir.AluOpType.bypass, replica_groups=[core_ids],
            ins=[slice1[:]], outs=[table1[:]])
        tc.strict_bb_all_engine_barrier()
        edge_phase(table1, er1_sb, 1)

    tc.strict_bb_all_engine_barrier()
    with tc.tile_pool(name="h1p", bufs=1) as h1p:
        h1T_sb = h1p.tile([128, ROWS], bf16)
        nc.sync.dma_start(h1T_sb[:], h1r[:], transpose=True)
        proj_phase(lambda nt: h1T_sb[:, nt * 128:(nt + 1) * 128],
                   w2_sb, slice2, er2_sb)
        tc.strict_bb_all_engine_barrier()
        nc.gpsimd.collective_compute(
            "AllGather", mybir.AluOpType.bypass, replica_groups=[core_ids],
            ins=[slice2[:]], outs=[table2[:]])
        tc.strict_bb_all_engine_barrier()
        edge_phase(table2, er2_sb, 2)


def build_nc(runs, T, compile=True):
    from concourse import bacc

    supmax = max(len(r["tiles"]) for r in runs)
    nc = bacc.Bacc("TRN2", target_bir_lowering=False, num_swdge_queues=4)
    with tile.TileContext(nc) as tc:
        build_kernel(tc, runs, T, supmax)
    if compile:
        nc.compile()
    return nc


def make_in_maps(per_core, feat, perms, W1, al1, ar1, b1, W2, al2, ar2, b2):
    w1a = aug_weights(np.asarray(W1, np.float32), np.asarray(al1, np.float32),
                      np.asarray(ar1, np.float32))
    w2a = aug_weights(np.asarray(W2, np.float32), np.asarray(al2, np.float32),
                      np.asarray(ar2, np.float32)).astype(ml_dtypes.bfloat16)
    iota = np.broadcast_to(np.arange(128, dtype=np.float32), (128, 128))
    iota = np.ascontiguousarray(iota.astype(ml_dtypes.bfloat16))
    sentel = np.full((1, 4), SENT_EL, np.float32).view(np.uint16).reshape(1, 8)
    sentel = sentel.view(ml_dtypes.bfloat16)
    b1r = np.ascontiguousarray(np.broadcast_to(
        np.asarray(b1, np.float32).reshape(1, F), (128, F)))
    b2m = np.asarray(b2, np.float32).reshape(H, OUT).mean(axis=0)
    b2r = np.ascontiguousarray(np.broadcast_to(b2m.reshape(1, OUT), (128, OUT)))
    feat = np.asarray(feat, np.float32)
    in_maps = []
    for c in range(NC):
        fs = np.zeros((ROWS, F), np.float32)
        fs[perms[c]] = feat[c * NPC:(c + 1) * NPC]
        m = dict(
            featT=np.ascontiguousarray(fs.T),
            w1aug=w1a, w2aug=w2a,
            osw=per_core[c]["osw"],
            dloc=per_core[c]["dloc"],
            iota=iota, b1rep=b1r, b2rep=b2r, sentel=sentel,
        )
        in_maps.append(m)
    return in_maps


_CACHE = {}


def _get_program(src, dst):
    per_core, perms, tb, runs, T = prep_inputs(src, dst)
    key = (T, tb.tobytes())
    if key not in _CACHE:
        _CACHE[key] = build_nc(runs, T)
    return _CACHE[key], per_core, perms


def kernel(feat, src, dst, W1, al1, ar1, b1, W2, al2, ar2, b2,
           _trace=False, _return_results=False):
    from concourse.bass_utils import run_bass_kernel_spmd

    nc, per_core, perms = _get_program(src, dst)
    in_maps = make_in_maps(per_core, feat, perms, W1, al1, ar1, b1,
                           W2, al2, ar2, b2)
    res = run_bass_kernel_spmd(nc, in_maps, list(range(NC)), trace=_trace)
    out = np.zeros((NC * NPC, OUT), np.float32)
    for c in range(NC):
        oc = np.asarray(res.results[c]["out"])
        out[c * NPC:(c + 1) * NPC] = oc[perms[c]]
    if _return_results:
        return out, res
    return out


# revision 12
# speedup vs baseline: 1.8769x; 1.4432x over previous
"""Two-layer GAT (DGL GATConv) on 8 Trainium2 NeuronCores via Bass/Tile.

v2: dst-partitioned graph parallel with dma_gather edge gathers.

  - Nodes are slot-relabeled per core (greedy block assignment balancing
    per-(block, src-window) edge counts); everything on device is in slot
    order, so both layers share one gather-index array and one one-hot
    structure.
  - Per layer: project own slice (h, el, er from one matmul vs an augmented
    weight matrix), pack 512B table rows [h0|1|h1|1|h2|1|h3|1 bf16 | el f32],
    AllGather the table, then gather per-edge src rows with int16 dma_gather
    (4 windows of 2 core-slices each keep indices < 32768), compute edge
    softmax and aggregate per 128-dst block with one bf16 matmul per tile.
    The interleaved "1" columns make the same matmul emit the per-dst softmax
    normalizers. er[dst] is produced on-chip per tile by transposing the
    one-hot on the PE array and multiplying with the SBUF-resident er table.
"""
import sys

sys.path.insert(0, "/opt/trn_rl_repo")

import math
from contextlib import ExitStack

import ml_dtypes
import numpy as np

import concourse.bass as bass
import concourse.mybir as mybir
import concourse.tile as tile
from concourse._compat import with_exitstack
from concourse.masks import make_identity

NEG = 0.2
F = 128
H = 4
OUT = 32
ROWB = 256          # bf16 elems per table row (512 B)
NC = 8
NPC = 12500
NB = 98             # 128-dst blocks per core
ROWS = NB * 128     # 12544 slots per core
SENT = ROWS - 1     # reserved pad slot on every core (block 97 capped at 127)
TROWS = ROWS * NC
WIN = 4
WROWS = 2 * ROWS    # rows per gather window (pair of core slices), < 32768
GROUP = 4           # dst blocks per PSUM accumulation group
SENT_EL = -80.0     # sentinel el -> exp(lrelu(-80+er)) ~ 1e-7

bf16 = mybir.dt.bfloat16
f32 = mybir.dt.float32
i16 = mybir.dt.int16


# ---------------------------------------------------------------- host prep

def _shared_structure(n3):
    """Per-(block, window) tile budgets + global tile ordering."""
    tb = np.full((NB, WIN), 2, np.int64)
    for w in range(WIN):
        big = (np.arange(n3) * NB // n3 + w * 7) % NB
        tb[np.unique(big), w] = 3
    groups = [list(range(g * GROUP, min(NB, (g + 1) * GROUP)))
              for g in range(math.ceil(NB / GROUP))]
    runs = []           # dict(w, t0, tiles=[(b, start, stop)], fin=[blocks])
    tiles_bw = [[None] * WIN for _ in range(NB)]
    t = 0
    for blocks in groups:
        for w in range(WIN):
            tl = []
            for b in blocks:
                tiles_bw[b][w] = np.arange(t + len(tl), t + len(tl) + tb[b, w])
                for k in range(tb[b, w]):
                    tl.append((b, w == 0 and k == 0,
                               w == WIN - 1 and k == tb[b, WIN - 1] - 1))
            runs.append(dict(w=w, t0=t, tiles=tl,
                             fin=blocks if w == WIN - 1 else []))
            t += len(tl)
    return tb, runs, tiles_bw, t


def _assign_blocks(wvec, tb):
    """Greedy: assign dsts (with per-window edge counts) to blocks under
    per-(b,w) capacity tb*128 and per-block dst capacity."""
    cap = tb * 128
    capd = np.full(NB, 128, np.int64)
    capd[NB - 1] = 127          # reserve SENT slot
    deg = wvec.sum(1)
    order = np.argsort(-deg, kind="stable")
    cnt = np.zeros((NB, WIN), np.int64)
    ndst = np.zeros(NB, np.int64)
    blk = np.empty(NPC, np.int64)
    slot_in = np.empty(NPC, np.int64)
    for d in order:
        resid = cap - cnt - wvec[d]
        ok = (resid.min(1) >= 0) & (ndst < capd)
        if not ok.any():
            return None, None
        score = np.where(ok, resid.min(1) * 1000 - ndst, -(10 ** 9))
        b = int(np.argmax(score))
        blk[d] = b
        slot_in[d] = ndst[b]
        cnt[b] += wvec[d]
        ndst[b] += 1
    return blk * 128 + slot_in, cnt


def prep_inputs(src, dst):
    src = np.asarray(src).astype(np.int64)
    dst = np.asarray(dst).astype(np.int64)
    win_edge = src // (2 * NPC)          # gather window of each edge (by src)

    n3 = 8
    while True:
        tb, runs, tiles_bw, T = _shared_structure(n3)
        perms = []
        ecore = []
        ok = True
        for c in range(NC):
            eid = np.nonzero((dst >= c * NPC) & (dst < (c + 1) * NPC))[0]
            d0 = dst[eid] - c * NPC
            wv = win_edge[eid]
            wvec = np.zeros((NPC, WIN), np.int64)
            np.add.at(wvec, (d0, wv), 1)
            perm, _ = _assign_blocks(wvec, tb)
            if perm is None:
                ok = False
                break
            perms.append(perm)
            ecore.append((eid, d0, wv))
        if ok:
            break
        n3 += 4
        assert n3 <= 32, "edge packing infeasible"

    rowof = np.empty(src.max() + 1 if False else NC * NPC, np.int64)
    for c in range(NC):
        rowof[c * NPC:(c + 1) * NPC] = c * ROWS + perms[c]

    per_core = []
    for c in range(NC):
        eid, d0, wv = ecore[c]
        perm = perms[c]
        osflat = np.full(T * 128, SENT, np.int32)
        dlflat = np.full(T * 128, 127, np.float32)
        b_e = perm[d0] // 128
        slot_e = (perm[d0] % 128).astype(np.float32)
        relrow = (rowof[src[eid]] - wv * WROWS).astype(np.int32)
        key = b_e * WIN + wv
        order_e = np.argsort(key, kind="stable")
        counts = np.bincount(key, minlength=NB * WIN)
        off = np.concatenate([[0], np.cumsum(counts)])
        for b in range(NB):
            for w in range(WIN):
                j0, j1 = off[b * WIN + w], off[b * WIN + w + 1]
                if j1 == j0:
                    continue
                es = order_e[j0:j1]
                jj = np.arange(j1 - j0)
                pos = tiles_bw[b][w][jj // 128] * 128 + jj % 128
                osflat[pos] = relrow[es]
                dlflat[pos] = slot_e[es]
        assert osflat.max() < WROWS and osflat.min() >= 0
        osw = np.tile(
            np.ascontiguousarray(
                osflat.astype(np.int16).reshape(T * 8, 16).T), (8, 1))
        dl = np.ascontiguousarray(
            dlflat.reshape(T, 128).T).astype(ml_dtypes.bfloat16)
        per_core.append(dict(osw=osw, dloc=dl))
    return per_core, perms, tb, runs, T


def aug_weights(W, al, ar):
    """[128, 136] f32: [W | W@al_h | W@ar_h]."""
    Wa = np.zeros((F, 136), np.float32)
    Wa[:, :F] = W
    for h in range(H):
        Wa[:, F + h] = W[:, h * OUT:(h + 1) * OUT] @ al[h]
        Wa[:, F + H + h] = W[:, h * OUT:(h + 1) * OUT] @ ar[h]
    return Wa


# ---------------------------------------------------------------- kernel

@with_exitstack
def build_kernel(ctx: ExitStack, tc: tile.TileContext, runs, T, supmax):
    nc = tc.nc

    featT = nc.dram_tensor("featT", [F, ROWS], f32, kind="ExternalInput")
    w1 = nc.dram_tensor("w1aug", [F, 136], f32, kind="ExternalInput")
    w2 = nc.dram_tensor("w2aug", [F, 136], bf16, kind="ExternalInput")
    osw_d = nc.dram_tensor("osw", [128, T * 8], i16, kind="ExternalInput")
    dloc_d = nc.dram_tensor("dloc", [128, T], bf16, kind="ExternalInput")
    iota_d = nc.dram_tensor("iota", [128, 128], bf16, kind="ExternalInput")
    b1_d = nc.dram_tensor("b1rep", [128, F], f32, kind="ExternalInput")
    b2_d = nc.dram_tensor("b2rep", [128, OUT], f32, kind="ExternalInput")
    sent_d = nc.dram_tensor("sentel", [1, 8], bf16, kind="ExternalInput")
    out_ext = nc.dram_tensor("out", [ROWS, OUT], f32, kind="ExternalOutput")

    slice1 = nc.dram_tensor("slice1", [ROWS, ROWB], bf16)
    slice2 = nc.dram_tensor("slice2", [ROWS, ROWB], bf16)
    table1 = nc.dram_tensor("table1", [TROWS, ROWB], bf16, addr_space="Shared")
    table2 = nc.dram_tensor("table2", [TROWS, ROWB], bf16, addr_space="Shared")
    h1r = nc.dram_tensor("h1r", [ROWS, F], bf16)

    core_ids = list(range(NC))

    consts = ctx.enter_context(tc.tile_pool(name="consts", bufs=1))
    offp = ctx.enter_context(tc.tile_pool(name="offp", bufs=1))
    erp = ctx.enter_context(tc.tile_pool(name="erp", bufs=1))

    w1_sb = consts.tile([F, 136], f32)
    w2_sb = consts.tile([F, 136], bf16)
    iota_sb = consts.tile([128, 128], bf16)
    b1_sb = consts.tile([128, F], f32)
    b2_sb = consts.tile([128, OUT], f32)
    ident = consts.tile([128, 128], bf16)
    sent_sb = consts.tile([1, 8], bf16)
    nc.sync.dma_start(w1_sb[:], w1[:])
    nc.sync.dma_start(w2_sb[:], w2[:])
    nc.sync.dma_start(iota_sb[:], iota_d[:])
    nc.sync.dma_start(b1_sb[:], b1_d[:])
    nc.sync.dma_start(b2_sb[:], b2_d[:])
    nc.sync.dma_start(sent_sb[:], sent_d[:])
    make_identity(nc, ident[:])

    osw_sb = offp.tile([128, T * 8], i16)
    dloc_sb = offp.tile([128, T], bf16)
    nc.sync.dma_start(osw_sb[:], osw_d[:])
    nc.sync.dma_start(dloc_sb[:], dloc_d[:])

    er1_sb = erp.tile([128, NB * 4], bf16)
    er2_sb = erp.tile([128, NB * 4], bf16)

    def proj_phase(get_lhsT, w_sb, slice_d, er_sb):
        with tc.tile_pool(name="php", bufs=2, space="PSUM") as php, \
             tc.tile_pool(name="projp", bufs=2) as projp:
            for nt in range(NB):
                ph = php.tile([128, 136], f32, tag="ph")
                nc.tensor.matmul(out=ph[:], lhsT=get_lhsT(nt), rhs=w_sb[:],
                                 start=True, stop=True)
                row_t = projp.tile([128, ROWB], bf16, tag="rowt")
                rv = row_t[:, :].bitcast(f32)
                # plain row: [h(128 bf16) | el f32 (cols 128..135) | zeros]
                nc.scalar.copy(row_t[:, 0:128], ph[:, 0:128])
                nc.vector.memset(row_t[:, 136:ROWB], 0.0)
                nc.vector.tensor_copy(out=rv[:, 64:68], in_=ph[:, 128:132])
                nc.vector.tensor_copy(out=er_sb[:, nt * 4:(nt + 1) * 4],
                                      in_=ph[:, 132:136])
                nc.sync.dma_start(slice_d[nt * 128:(nt + 1) * 128, :], row_t[:])
            tc.strict_bb_all_engine_barrier()
            # sentinel slot: el := -80 (bf16 cols 128:136 = f32 el field)
            nc.sync.dma_start(slice_d[SENT:SENT + 1, 128:136], sent_sb[:])

    def edge_phase(table, er_sb, layer):
        with tc.tile_pool(name="accp", bufs=GROUP, space="PSUM") as accp, \
             tc.tile_pool(name="tpp", bufs=2, space="PSUM") as tpp, \
             tc.tile_pool(name="erps", bufs=2, space="PSUM") as erps, \
             tc.tile_pool(name="gp", bufs=3) as gp, \
             tc.tile_pool(name="ep", bufs=3) as ep, \
             tc.tile_pool(name="outp", bufs=2) as outp:
            acc_t = {}
            qn = [0]
            for run in runs:
                w, t0, tiles = run["w"], run["t0"], run["tiles"]
                n = len(tiles)
                g_t = gp.tile([128, supmax, ROWB], bf16, tag="g")
                # Q7 dma_gather caps at 1024 indices per call; spread
                # calls over the 4 SWDGE queues so SDMA engines interleave
                # packets (more outstanding HBM reads).
                for c0 in range(0, n, 4):
                    cn = min(4, n - c0)
                    nc.gpsimd.dma_gather(
                        g_t[:, c0:c0 + cn, :],
                        table[w * WROWS:(w + 1) * WROWS, :],
                        osw_sb[:, (t0 + c0) * 8:(t0 + c0 + cn) * 8],
                        cn * 128, cn * 128, ROWB,
                        single_packet=False,
                        queue_num=(qn[0] % 4))
                    qn[0] += 1

                s0_t = ep.tile([128, supmax * 128], bf16, tag="s0")
                io = iota_sb[:, :]
                dl = dloc_sb[:, t0:t0 + n]
                iota_ap = bass.AP(tensor=io.tensor, offset=io.offset,
                                  ap=[io.ap[0], [0, n], [1, 128]])
                dloc_ap = bass.AP(tensor=dl.tensor, offset=dl.offset,
                                  ap=[dl.ap[0], [1, n], [0, 128]])
                nc.vector.tensor_tensor(out=s0_t[:, :n * 128], in0=iota_ap,
                                        in1=dloc_ap,
                                        op=mybir.AluOpType.is_equal)

                er_ps = erps.tile([128, supmax * 4], f32, tag="erps")
                for k, (b, st, sp) in enumerate(tiles):
                    s0T_ps = tpp.tile([128, 128], bf16, tag="s0T")
                    nc.tensor.transpose(s0T_ps[:],
                                        s0_t[:, k * 128:(k + 1) * 128],
                                        ident[:])
                    s0T_sb = ep.tile([128, 128], bf16, tag="s0Ts")
                    nc.scalar.copy(s0T_sb[:], s0T_ps[:])
                    nc.tensor.matmul(out=er_ps[:, k * 4:(k + 1) * 4],
                                     lhsT=s0T_sb[:],
                                     rhs=er_sb[:, b * 4:(b + 1) * 4],
                                     start=True, stop=True,
                                     skip_group_check=True)

                g32 = g_t[:, :, :].bitcast(f32)
                el_ap = bass.AP(tensor=g32.tensor, offset=g32.offset + 64,
                                ap=[g32.ap[0], [128, n], [1, 4]])
                lg = ep.tile([128, supmax * 4], f32, tag="lg")
                nc.vector.tensor_tensor(out=lg[:, :n * 4], in0=el_ap,
                                        in1=er_ps[:, :n * 4],
                                        op=mybir.AluOpType.add)
                lrl = ep.tile([128, supmax * 4], f32, tag="lrl")
                nc.vector.tensor_scalar_mul(lrl[:, :n * 4], lg[:, :n * 4], NEG)
                nc.vector.tensor_tensor(out=lrl[:, :n * 4], in0=lg[:, :n * 4],
                                        in1=lrl[:, :n * 4],
                                        op=mybir.AluOpType.max)
                p_t = ep.tile([128, supmax * 4], bf16, tag="p")
                nc.scalar.activation(p_t[:, :n * 4], lrl[:, :n * 4],
                                     mybir.ActivationFunctionType.Exp)

                rhs_t = ep.tile([128, supmax * 132], bf16, tag="rhs")
                gb, pb, rb = g_t[:, :, :], p_t[:, :], rhs_t[:, :]
                for hd in range(H):
                    in0 = bass.AP(tensor=gb.tensor, offset=gb.offset + hd * 32,
                                  ap=[gb.ap[0], [ROWB, n], [1, 32]])
                    in1 = bass.AP(tensor=pb.tensor, offset=pb.offset + hd,
                                  ap=[pb.ap[0], [4, n], [0, 32]])
                    o = bass.AP(tensor=rb.tensor, offset=rb.offset + hd * 32,
                                ap=[rb.ap[0], [132, n], [1, 32]])
                    nc.vector.tensor_tensor(out=o, in0=in0, in1=in1,
                                            op=mybir.AluOpType.mult)
                pco = bass.AP(tensor=rb.tensor, offset=rb.offset + 128,
                              ap=[rb.ap[0], [132, n], [1, 4]])
                pci = bass.AP(tensor=pb.tensor, offset=pb.offset,
                              ap=[pb.ap[0], [4, n], [1, 4]])
                nc.vector.tensor_copy(out=pco, in_=pci)

                for k, (b, st, sp) in enumerate(tiles):
                    if st:
                        acc_t[b] = accp.tile([128, 132], f32, tag="acc",
                                             name=f"acc_l{layer}_b{b}")
                    nc.tensor.matmul(out=acc_t[b][:],
                                     lhsT=s0_t[:, k * 128:(k + 1) * 128],
                                     rhs=rhs_t[:, k * 132:(k + 1) * 132],
                                     start=st, stop=sp, skip_group_check=True)

                if run["fin"]:
                    finalize(run["fin"], acc_t, layer, outp)

    def finalize(blocks, acc_t, layer, outp):
        nb = len(blocks)
        stag = outp.tile([128, GROUP * 132], f32, tag="stag")
        for j, b in enumerate(blocks):
            nc.scalar.copy(stag[:, j * 132:(j + 1) * 132], acc_t[b][:])
        sv = stag[:, :]
        s_ap = bass.AP(tensor=sv.tensor, offset=sv.offset + 128,
                       ap=[sv.ap[0], [132, nb], [1, 4]])
        seps = outp.tile([128, GROUP * 4], f32, tag="seps")
        nc.vector.tensor_scalar_add(seps[:, :nb * 4], s_ap, 1e-30)
        inv = outp.tile([128, GROUP * 4], f32, tag="inv")
        nc.vector.reciprocal(inv[:, :nb * 4], seps[:, :nb * 4])
        if layer == 2:
            nc.vector.tensor_scalar_mul(inv[:, :nb * 4], inv[:, :nb * 4], 0.25)
        of = outp.tile([128, GROUP * 128], f32, tag="of")
        ov, iv = of[:, :], inv[:, :]
        for hd in range(H):
            msg_ap = bass.AP(tensor=sv.tensor, offset=sv.offset + hd * 32,
                             ap=[sv.ap[0], [132, nb], [1, 32]])
            inv_ap = bass.AP(tensor=iv.tensor, offset=iv.offset + hd,
                             ap=[iv.ap[0], [4, nb], [0, 32]])
            of_ap = bass.AP(tensor=ov.tensor, offset=ov.offset + hd * 32,
                            ap=[ov.ap[0], [128, nb], [1, 32]])
            nc.vector.tensor_tensor(out=of_ap, in0=msg_ap, in1=inv_ap,
                                    op=mybir.AluOpType.mult)
        if layer == 1:
            bv = b1_sb[:, :]
            b1_ap = bass.AP(tensor=bv.tensor, offset=bv.offset,
                            ap=[bv.ap[0], [0, nb], [1, F]])
            of2_ap = bass.AP(tensor=ov.tensor, offset=ov.offset,
                             ap=[ov.ap[0], [128, nb], [1, F]])
            nc.vector.tensor_tensor(out=of2_ap, in0=of2_ap, in1=b1_ap,
                                    op=mybir.AluOpType.add)
            h1row = outp.tile([128, GROUP * 128], bf16, tag="h1row")
            nc.scalar.activation(h1row[:, :nb * 128], of[:, :nb * 128],
                                 mybir.ActivationFunctionType.Relu)
            for j, b in enumerate(blocks):
                nc.sync.dma_start(h1r[b * 128:(b + 1) * 128, :],
                                  h1row[:, j * 128:(j + 1) * 128])
        else:
            # mean over heads = ((h0+h1) + (h2+h3)) with 0.25 folded into inv
            rd = outp.tile([128, GROUP * OUT], f32, tag="rd")
            rd2 = outp.tile([128, GROUP * OUT], f32, tag="rd2")
            aps = [bass.AP(tensor=ov.tensor, offset=ov.offset + hd * 32,
                           ap=[ov.ap[0], [128, nb], [1, 32]]) for hd in range(H)]
            rda = bass.AP(tensor=rd[:, :].tensor, offset=rd[:, :].offset,
                          ap=[rd[:, :].ap[0], [32, nb], [1, 32]])
            rda2 = bass.AP(tensor=rd2[:, :].tensor, offset=rd2[:, :].offset,
                           ap=[rd2[:, :].ap[0], [32, nb], [1, 32]])
            nc.vector.tensor_tensor(out=rda, in0=aps[0], in1=aps[1],
                                    op=mybir.AluOpType.add)
            nc.vector.tensor_tensor(out=rda2, in0=aps[2], in1=aps[3],
                                    op=mybir.AluOpType.add)
            nc.vector.tensor_tensor(out=rda, in0=rda, in1=rda2,
                                    op=mybir.AluOpType.add)
            bv2 = b2_sb[:, :]
            b2_ap = bass.AP(tensor=bv2.tensor, offset=bv2.offset,
                            ap=[bv2.ap[0], [0, nb], [1, OUT]])
            nc.vector.tensor_tensor(out=rda, in0=rda, in1=b2_ap,
                                    op=mybir.AluOpType.add)
            for j, b in enumerate(blocks):
                nc.sync.dma_start(out_ext[b * 128:(b + 1) * 128, :],
                                  rd[:, j * OUT:(j + 1) * OUT])

    # ---------------- phase sequence ----------------
    with tc.tile_pool(name="featp", bufs=1) as featp:
        featT_sb = featp.tile([F, ROWS], f32)
        nc.sync.dma_start(featT_sb[:], featT[:])
        proj_phase(lambda nt: featT_sb[:, nt * 128:(nt + 1) * 128],
                   w1_sb, slice1, er1_sb)
        tc.strict_bb_all_engine_barrier()
        nc.gpsimd.collective_compute(
            "AllGather", mybir.AluOpType.bypass, replica_groups=[core_ids],
            ins=[slice1[:]], outs=[table1[:]])
        tc.strict_bb_all_engine_barrier()
        edge_phase(table1, er1_sb, 1)

    tc.strict_bb_all_engine_barrier()
    with tc.tile_pool(name="h1p", bufs=1) as h1p:
        h1T_sb = h1p.tile([128, ROWS], bf16)
        nc.sync.dma_start(h1T_sb[:], h1r[:], transpose=True)
        proj_phase(lambda nt: h1T_sb[:, nt * 128:(nt + 1) * 128],
                   w2_sb, slice2, er2_sb)
        tc.strict_bb_all_engine_barrier()
        nc.gpsimd.collective_compute(
            "AllGather", mybir.AluOpType.bypass, replica_groups=[core_ids],
            ins=[slice2[:]], outs=[table2[:]])
        tc.strict_bb_all_engine_barrier()
        edge_phase(table2, er2_sb, 2)


def build_nc(runs, T, compile=True):
    from concourse import bacc

    supmax = max(len(r["tiles"]) for r in runs)
    nc = bacc.Bacc("TRN2", target_bir_lowering=False, num_swdge_queues=4)
    with tile.TileContext(nc) as tc:
        build_kernel(tc, runs, T, supmax)
    if compile:
        nc.compile()
    return nc


def make_in_maps(per_core, feat, perms, W1, al1, ar1, b1, W2, al2, ar2, b2):
    w1a = aug_weights(np.asarray(W1, np.float32), np.asarray(al1, np.float32),
                      np.asarray(ar1, np.float32))
    w2a = aug_weights(np.asarray(W2, np.float32), np.asarray(al2, np.float32),
                      np.asarray(ar2, np.float32)).astype(ml_dtypes.bfloat16)
    iota = np.broadcast_to(np.arange(128, dtype=np.float32), (128, 128))
    iota = np.ascontiguousarray(iota.astype(ml_dtypes.bfloat16))
    sentel = np.full((1, 4), SENT_EL, np.float32).view(np.uint16).reshape(1, 8)
    sentel = sentel.view(ml_dtypes.bfloat16)
    b1r = np.ascontiguousarray(np.broadcast_to(
        np.asarray(b1, np.float32).reshape(1, F), (128, F)))
    b2m = np.asarray(b2, np.float32).reshape(H, OUT).mean(axis=0)
    b2r = np.ascontiguousarray(np.broadcast_to(b2m.reshape(1, OUT), (128, OUT)))
    feat = np.asarray(feat, np.float32)
    in_maps = []
    for c in range(NC):
        fs = np.zeros((ROWS, F), np.float32)
        fs[perms[c]] = feat[c * NPC:(c + 1) * NPC]
        m = dict(
            featT=np.ascontiguousarray(fs.T),
            w1aug=w1a, w2aug=w2a,
            osw=per_core[c]["osw"],
            dloc=per_core[c]["dloc"],
            iota=iota, b1rep=b1r, b2rep=b2r, sentel=sentel,
        )
        in_maps.append(m)
    return in_maps


_CACHE = {}


def _get_program(src, dst):
    per_core, perms, tb, runs, T = prep_inputs(src, dst)
    key = (T, tb.tobytes())
    if key not in _CACHE:
        _CACHE[key] = build_nc(runs, T)
    return _CACHE[key], per_core, perms


def kernel(feat, src, dst, W1, al1, ar1, b1, W2, al2, ar2, b2,
           _trace=False, _return_results=False):
    from concourse.bass_utils import run_bass_kernel_spmd

    nc, per_core, perms = _get_program(src, dst)
    in_maps = make_in_maps(per_core, feat, perms, W1, al1, ar1, b1,
                           W2, al2, ar2, b2)
    res = run_bass_kernel_spmd(nc, in_maps, list(range(NC)), trace=_trace)
    out = np.zeros((NC * NPC, OUT), np.float32)
    for c in range(NC):
        oc = np.asarray(res.results[c]["out"])
        out[c * NPC:(c + 1) * NPC] = oc[perms[c]]
    if _return_results:
        return out, res
    return out


# revision 14
# speedup vs baseline: 1.8777x; 1.0004x over previous
"""Two-layer GAT (DGL GATConv) on 8 Trainium2 NeuronCores via Bass/Tile.

v2: dst-partitioned graph parallel with dma_gather edge gathers.

  - Nodes are slot-relabeled per core (greedy block assignment balancing
    per-(block, src-window) edge counts); everything on device is in slot
    order, so both layers share one gather-index array and one one-hot
    structure.
  - Per layer: project own slice (h, el, er from one matmul vs an augmented
    weight matrix), pack 512B table rows [h0|1|h1|1|h2|1|h3|1 bf16 | el f32],
    AllGather the table, then gather per-edge src rows with int16 dma_gather
    (4 windows of 2 core-slices each keep indices < 32768), compute edge
    softmax and aggregate per 128-dst block with one bf16 matmul per tile.
    The interleaved "1" columns make the same matmul emit the per-dst softmax
    normalizers. er[dst] is produced on-chip per tile by transposing the
    one-hot on the PE array and multiplying with the SBUF-resident er table.
"""
import sys

sys.path.insert(0, "/opt/trn_rl_repo")

import math
from contextlib import ExitStack

import ml_dtypes
import numpy as np

import concourse.bass as bass
import concourse.mybir as mybir
import concourse.tile as tile
from concourse._compat import with_exitstack
from concourse.masks import make_identity

NEG = 0.2
F = 128
H = 4
OUT = 32
ROWB = 256          # bf16 elems per table row (512 B)
NC = 8
NPC = 12500
NB = 98             # 128-dst blocks per core
ROWS = NB * 128     # 12544 slots per core
SENT = ROWS - 1     # reserved pad slot on every core (block 97 capped at 127)
TROWS = ROWS * NC
WIN = 4
WROWS = 2 * ROWS    # rows per gather window (pair of core slices), < 32768
GROUP = 4           # dst blocks per PSUM accumulation group
SENT_EL = -80.0     # sentinel el -> exp(lrelu(-80+er)) ~ 1e-7

bf16 = mybir.dt.bfloat16
f32 = mybir.dt.float32
i16 = mybir.dt.int16


# ---------------------------------------------------------------- host prep

def _shared_structure(n3):
    """Per-(block, window) tile budgets + global tile ordering."""
    tb = np.full((NB, WIN), 2, np.int64)
    for w in range(WIN):
        big = (np.arange(n3) * NB // n3 + w * 7) % NB
        tb[np.unique(big), w] = 3
    groups = [list(range(g * GROUP, min(NB, (g + 1) * GROUP)))
              for g in range(math.ceil(NB / GROUP))]
    runs = []           # dict(w, t0, tiles=[(b, start, stop)], fin=[blocks])
    tiles_bw = [[None] * WIN for _ in range(NB)]
    t = 0
    for blocks in groups:
        for w in range(WIN):
            tl = []
            for b in blocks:
                tiles_bw[b][w] = np.arange(t + len(tl), t + len(tl) + tb[b, w])
                for k in range(tb[b, w]):
                    tl.append((b, w == 0 and k == 0,
                               w == WIN - 1 and k == tb[b, WIN - 1] - 1))
            runs.append(dict(w=w, t0=t, tiles=tl,
                             fin=blocks if w == WIN - 1 else []))
            t += len(tl)
    return tb, runs, tiles_bw, t


def _assign_blocks(wvec, tb):
    """Greedy: assign dsts (with per-window edge counts) to blocks under
    per-(b,w) capacity tb*128 and per-block dst capacity."""
    cap = tb * 128
    capd = np.full(NB, 128, np.int64)
    capd[NB - 1] = 127          # reserve SENT slot
    deg = wvec.sum(1)
    order = np.argsort(-deg, kind="stable")
    cnt = np.zeros((NB, WIN), np.int64)
    ndst = np.zeros(NB, np.int64)
    blk = np.empty(NPC, np.int64)
    slot_in = np.empty(NPC, np.int64)
    for d in order:
        resid = cap - cnt - wvec[d]
        ok = (resid.min(1) >= 0) & (ndst < capd)
        if not ok.any():
            return None, None
        score = np.where(ok, resid.min(1) * 1000 - ndst, -(10 ** 9))
        b = int(np.argmax(score))
        blk[d] = b
        slot_in[d] = ndst[b]
        cnt[b] += wvec[d]
        ndst[b] += 1
    return blk * 128 + slot_in, cnt


def prep_inputs(src, dst):
    src = np.asarray(src).astype(np.int64)
    dst = np.asarray(dst).astype(np.int64)
    win_edge = src // (2 * NPC)          # gather window of each edge (by src)

    n3 = 8
    while True:
        tb, runs, tiles_bw, T = _shared_structure(n3)
        perms = []
        ecore = []
        ok = True
        for c in range(NC):
            eid = np.nonzero((dst >= c * NPC) & (dst < (c + 1) * NPC))[0]
            d0 = dst[eid] - c * NPC
            wv = win_edge[eid]
            wvec = np.zeros((NPC, WIN), np.int64)
            np.add.at(wvec, (d0, wv), 1)
            perm, _ = _assign_blocks(wvec, tb)
            if perm is None:
                ok = False
                break
            perms.append(perm)
            ecore.append((eid, d0, wv))
        if ok:
            break
        n3 += 4
        assert n3 <= 32, "edge packing infeasible"

    rowof = np.empty(src.max() + 1 if False else NC * NPC, np.int64)
    for c in range(NC):
        rowof[c * NPC:(c + 1) * NPC] = c * ROWS + perms[c]

    per_core = []
    for c in range(NC):
        eid, d0, wv = ecore[c]
        perm = perms[c]
        osflat = np.full(T * 128, SENT, np.int32)
        dlflat = np.full(T * 128, 127, np.float32)
        b_e = perm[d0] // 128
        slot_e = (perm[d0] % 128).astype(np.float32)
        relrow = (rowof[src[eid]] - wv * WROWS).astype(np.int32)
        key = b_e * WIN + wv
        order_e = np.argsort(key, kind="stable")
        counts = np.bincount(key, minlength=NB * WIN)
        off = np.concatenate([[0], np.cumsum(counts)])
        for b in range(NB):
            for w in range(WIN):
                j0, j1 = off[b * WIN + w], off[b * WIN + w + 1]
                if j1 == j0:
                    continue
                es = order_e[j0:j1]
                jj = np.arange(j1 - j0)
                pos = tiles_bw[b][w][jj // 128] * 128 + jj % 128
                osflat[pos] = relrow[es]
                dlflat[pos] = slot_e[es]
        assert osflat.max() < WROWS and osflat.min() >= 0
        osw = np.tile(
            np.ascontiguousarray(
                osflat.astype(np.int16).reshape(T * 8, 16).T), (8, 1))
        dl = np.ascontiguousarray(
            dlflat.reshape(T, 128).T).astype(ml_dtypes.bfloat16)
        per_core.append(dict(osw=osw, dloc=dl))
    return per_core, perms, tb, runs, T


def aug_weights(W, al, ar):
    """[128, 136] f32: [W | W@al_h | W@ar_h]."""
    Wa = np.zeros((F, 136), np.float32)
    Wa[:, :F] = W
    for h in range(H):
        Wa[:, F + h] = W[:, h * OUT:(h + 1) * OUT] @ al[h]
        Wa[:, F + H + h] = W[:, h * OUT:(h + 1) * OUT] @ ar[h]
    return Wa


# ---------------------------------------------------------------- kernel

@with_exitstack
def build_kernel(ctx: ExitStack, tc: tile.TileContext, runs, T, supmax):
    nc = tc.nc

    featT = nc.dram_tensor("featT", [F, ROWS], f32, kind="ExternalInput")
    w1 = nc.dram_tensor("w1aug", [F, 136], f32, kind="ExternalInput")
    w2 = nc.dram_tensor("w2aug", [F, 136], bf16, kind="ExternalInput")
    osw_d = nc.dram_tensor("osw", [128, T * 8], i16, kind="ExternalInput")
    dloc_d = nc.dram_tensor("dloc", [128, T], bf16, kind="ExternalInput")
    iota_d = nc.dram_tensor("iota", [128, 128], bf16, kind="ExternalInput")
    b1_d = nc.dram_tensor("b1rep", [128, F], f32, kind="ExternalInput")
    b2_d = nc.dram_tensor("b2rep", [128, OUT], f32, kind="ExternalInput")
    sent_d = nc.dram_tensor("sentel", [1, 8], bf16, kind="ExternalInput")
    out_ext = nc.dram_tensor("out", [ROWS, OUT], f32, kind="ExternalOutput")

    slice1 = nc.dram_tensor("slice1", [ROWS, ROWB], bf16)
    slice2 = nc.dram_tensor("slice2", [ROWS, ROWB], bf16)
    table1 = nc.dram_tensor("table1", [TROWS, ROWB], bf16, addr_space="Shared")
    table2 = nc.dram_tensor("table2", [TROWS, ROWB], bf16, addr_space="Shared")
    h1r = nc.dram_tensor("h1r", [ROWS, F], bf16)

    core_ids = list(range(NC))

    consts = ctx.enter_context(tc.tile_pool(name="consts", bufs=1))
    offp = ctx.enter_context(tc.tile_pool(name="offp", bufs=1))
    erp = ctx.enter_context(tc.tile_pool(name="erp", bufs=1))

    w1_sb = consts.tile([F, 136], f32)
    w2_sb = consts.tile([F, 136], bf16)
    iota_sb = consts.tile([128, 128], bf16)
    b1_sb = consts.tile([128, F], f32)
    b2_sb = consts.tile([128, OUT], f32)
    ident = consts.tile([128, 128], bf16)
    sent_sb = consts.tile([1, 8], bf16)
    nc.sync.dma_start(w1_sb[:], w1[:])
    nc.sync.dma_start(w2_sb[:], w2[:])
    nc.sync.dma_start(iota_sb[:], iota_d[:])
    nc.sync.dma_start(b1_sb[:], b1_d[:])
    nc.sync.dma_start(b2_sb[:], b2_d[:])
    nc.sync.dma_start(sent_sb[:], sent_d[:])
    make_identity(nc, ident[:])

    osw_sb = offp.tile([128, T * 8], i16)
    dloc_sb = offp.tile([128, T], bf16)
    nc.sync.dma_start(osw_sb[:], osw_d[:])
    nc.sync.dma_start(dloc_sb[:], dloc_d[:])

    er1_sb = erp.tile([128, NB * 4], bf16)
    er2_sb = erp.tile([128, NB * 4], bf16)

    def proj_phase(get_lhsT, w_sb, slice_d, er_sb):
        with tc.tile_pool(name="php", bufs=2, space="PSUM") as php, \
             tc.tile_pool(name="projp", bufs=2) as projp:
            for nt in range(NB):
                ph = php.tile([128, 136], f32, tag="ph")
                nc.tensor.matmul(out=ph[:], lhsT=get_lhsT(nt), rhs=w_sb[:],
                                 start=True, stop=True)
                row_t = projp.tile([128, ROWB], bf16, tag="rowt")
                rv = row_t[:, :].bitcast(f32)
                # plain row: [h(128 bf16) | el f32 (cols 128..135) | zeros]
                nc.scalar.copy(row_t[:, 0:128], ph[:, 0:128])
                nc.vector.memset(row_t[:, 136:ROWB], 0.0)
                nc.vector.tensor_copy(out=rv[:, 64:68], in_=ph[:, 128:132])
                nc.vector.tensor_copy(out=er_sb[:, nt * 4:(nt + 1) * 4],
                                      in_=ph[:, 132:136])
                nc.sync.dma_start(slice_d[nt * 128:(nt + 1) * 128, :], row_t[:])
            tc.strict_bb_all_engine_barrier()
            # sentinel slot: el := -80 (bf16 cols 128:136 = f32 el field)
            nc.sync.dma_start(slice_d[SENT:SENT + 1, 128:136], sent_sb[:])

    def edge_phase(table, er_sb, layer):
        with tc.tile_pool(name="accp", bufs=GROUP, space="PSUM") as accp, \
             tc.tile_pool(name="tpp", bufs=2, space="PSUM") as tpp, \
             tc.tile_pool(name="erps", bufs=2, space="PSUM") as erps, \
             tc.tile_pool(name="gp", bufs=3) as gp, \
             tc.tile_pool(name="ep", bufs=3) as ep, \
             tc.tile_pool(name="outp", bufs=2) as outp:
            acc_t = {}
            qn = [0]
            for run in runs:
                w, t0, tiles = run["w"], run["t0"], run["tiles"]
                n = len(tiles)
                g_t = gp.tile([128, supmax, ROWB], bf16, tag="g")
                # Q7 dma_gather caps at 1024 indices per call; spread
                # calls over the 4 SWDGE queues so SDMA engines interleave
                # packets (more outstanding HBM reads).
                for c0 in range(0, n, 4):
                    cn = min(4, n - c0)
                    nc.gpsimd.dma_gather(
                        g_t[:, c0:c0 + cn, :],
                        table[w * WROWS:(w + 1) * WROWS, :],
                        osw_sb[:, (t0 + c0) * 8:(t0 + c0 + cn) * 8],
                        cn * 128, cn * 128, ROWB,
                        single_packet=False,
                        queue_num=(qn[0] % 4))
                    qn[0] += 1

                s0_t = ep.tile([128, supmax * 128], bf16, tag="s0")
                io = iota_sb[:, :]
                dl = dloc_sb[:, t0:t0 + n]
                iota_ap = bass.AP(tensor=io.tensor, offset=io.offset,
                                  ap=[io.ap[0], [0, n], [1, 128]])
                dloc_ap = bass.AP(tensor=dl.tensor, offset=dl.offset,
                                  ap=[dl.ap[0], [1, n], [0, 128]])
                nc.vector.tensor_tensor(out=s0_t[:, :n * 128], in0=iota_ap,
                                        in1=dloc_ap,
                                        op=mybir.AluOpType.is_equal)

                er_ps = erps.tile([128, supmax * 4], f32, tag="erps")
                for k, (b, st, sp) in enumerate(tiles):
                    s0T_ps = tpp.tile([128, 128], bf16, tag="s0T")
                    nc.tensor.transpose(s0T_ps[:],
                                        s0_t[:, k * 128:(k + 1) * 128],
                                        ident[:])
                    s0T_sb = ep.tile([128, 128], bf16, tag="s0Ts")
                    nc.scalar.copy(s0T_sb[:], s0T_ps[:])
                    nc.tensor.matmul(out=er_ps[:, k * 4:(k + 1) * 4],
                                     lhsT=s0T_sb[:],
                                     rhs=er_sb[:, b * 4:(b + 1) * 4],
                                     start=True, stop=True,
                                     skip_group_check=True)

                g32 = g_t[:, :, :].bitcast(f32)
                el_ap = bass.AP(tensor=g32.tensor, offset=g32.offset + 64,
                                ap=[g32.ap[0], [128, n], [1, 4]])
                lg = ep.tile([128, supmax * 4], f32, tag="lg")
                nc.vector.tensor_tensor(out=lg[:, :n * 4], in0=el_ap,
                                        in1=er_ps[:, :n * 4],
                                        op=mybir.AluOpType.add)
                lrl = ep.tile([128, supmax * 4], f32, tag="lrl")
                nc.vector.tensor_scalar_mul(lrl[:, :n * 4], lg[:, :n * 4], NEG)
                nc.vector.tensor_tensor(out=lrl[:, :n * 4], in0=lg[:, :n * 4],
                                        in1=lrl[:, :n * 4],
                                        op=mybir.AluOpType.max)
                p_t = ep.tile([128, supmax * 4], bf16, tag="p")
                nc.scalar.activation(p_t[:, :n * 4], lrl[:, :n * 4],
                                     mybir.ActivationFunctionType.Exp)

                rhs_t = ep.tile([128, supmax * 132], bf16, tag="rhs")
                gb, pb, rb = g_t[:, :, :], p_t[:, :], rhs_t[:, :]
                for hd in range(H):
                    in0 = bass.AP(tensor=gb.tensor, offset=gb.offset + hd * 32,
                                  ap=[gb.ap[0], [ROWB, n], [1, 32]])
                    in1 = bass.AP(tensor=pb.tensor, offset=pb.offset + hd,
                                  ap=[pb.ap[0], [4, n], [0, 32]])
                    o = bass.AP(tensor=rb.tensor, offset=rb.offset + hd * 32,
                                ap=[rb.ap[0], [132, n], [1, 32]])
                    nc.vector.tensor_tensor(out=o, in0=in0, in1=in1,
                                            op=mybir.AluOpType.mult)
                pco = bass.AP(tensor=rb.tensor, offset=rb.offset + 128,
                              ap=[rb.ap[0], [132, n], [1, 4]])
                pci = bass.AP(tensor=pb.tensor, offset=pb.offset,
                              ap=[pb.ap[0], [4, n], [1, 4]])
                nc.vector.tensor_copy(out=pco, in_=pci)

                for k, (b, st, sp) in enumerate(tiles):
                    if st:
                        acc_t[b] = accp.tile([128, 132], f32, tag="acc",
                                             name=f"acc_l{layer}_b{b}")
                    nc.tensor.matmul(out=acc_t[b][:],
                                     lhsT=s0_t[:, k * 128:(k + 1) * 128],
                                     rhs=rhs_t[:, k * 132:(k + 1) * 132],
                                     start=st, stop=sp, skip_group_check=True)

                if run["fin"]:
                    finalize(run["fin"], acc_t, layer, outp)

    def finalize(blocks, acc_t, layer, outp):
        nb = len(blocks)
        stag = outp.tile([128, GROUP * 132], f32, tag="stag")
        for j, b in enumerate(blocks):
            nc.scalar.copy(stag[:, j * 132:(j + 1) * 132], acc_t[b][:])
        sv = stag[:, :]
        s_ap = bass.AP(tensor=sv.tensor, offset=sv.offset + 128,
                       ap=[sv.ap[0], [132, nb], [1, 4]])
        seps = outp.tile([128, GROUP * 4], f32, tag="seps")
        nc.vector.tensor_scalar_add(seps[:, :nb * 4], s_ap, 1e-30)
        inv = outp.tile([128, GROUP * 4], f32, tag="inv")
        nc.vector.reciprocal(inv[:, :nb * 4], seps[:, :nb * 4])
        if layer == 2:
            nc.vector.tensor_scalar_mul(inv[:, :nb * 4], inv[:, :nb * 4], 0.25)
        of = outp.tile([128, GROUP * 128], f32, tag="of")
        ov, iv = of[:, :], inv[:, :]
        for hd in range(H):
            msg_ap = bass.AP(tensor=sv.tensor, offset=sv.offset + hd * 32,
                             ap=[sv.ap[0], [132, nb], [1, 32]])
            inv_ap = bass.AP(tensor=iv.tensor, offset=iv.offset + hd,
                             ap=[iv.ap[0], [4, nb], [0, 32]])
            of_ap = bass.AP(tensor=ov.tensor, offset=ov.offset + hd * 32,
                            ap=[ov.ap[0], [128, nb], [1, 32]])
            nc.vector.tensor_tensor(out=of_ap, in0=msg_ap, in1=inv_ap,
                                    op=mybir.AluOpType.mult)
        if layer == 1:
            bv = b1_sb[:, :]
            b1_ap = bass.AP(tensor=bv.tensor, offset=bv.offset,
                            ap=[bv.ap[0], [0, nb], [1, F]])
            of2_ap = bass.AP(tensor=ov.tensor, offset=ov.offset,
                             ap=[ov.ap[0], [128, nb], [1, F]])
            nc.vector.tensor_tensor(out=of2_ap, in0=of2_ap, in1=b1_ap,
                                    op=mybir.AluOpType.add)
            h1row = outp.tile([128, GROUP * 128], bf16, tag="h1row")
            nc.scalar.activation(h1row[:, :nb * 128], of[:, :nb * 128],
                                 mybir.ActivationFunctionType.Relu)
            for j, b in enumerate(blocks):
                nc.sync.dma_start(h1r[b * 128:(b + 1) * 128, :],
                                  h1row[:, j * 128:(j + 1) * 128])
        else:
            # mean over heads = ((h0+h1) + (h2+h3)) with 0.25 folded into inv
            rd = outp.tile([128, GROUP * OUT], f32, tag="rd")
            rd2 = outp.tile([128, GROUP * OUT], f32, tag="rd2")
            aps = [bass.AP(tensor=ov.tensor, offset=ov.offset + hd * 32,
                           ap=[ov.ap[0], [128, nb], [1, 32]]) for hd in range(H)]
            rda = bass.AP(tensor=rd[:, :].tensor, offset=rd[:, :].offset,
                          ap=[rd[:, :].ap[0], [32, nb], [1, 32]])
            rda2 = bass.AP(tensor=rd2[:, :].tensor, offset=rd2[:, :].offset,
                           ap=[rd2[:, :].ap[0], [32, nb], [1, 32]])
            nc.vector.tensor_tensor(out=rda, in0=aps[0], in1=aps[1],
                                    op=mybir.AluOpType.add)
            nc.vector.tensor_tensor(out=rda2, in0=aps[2], in1=aps[3],
                                    op=mybir.AluOpType.add)
            nc.vector.tensor_tensor(out=rda, in0=rda, in1=rda2,
                                    op=mybir.AluOpType.add)
            bv2 = b2_sb[:, :]
            b2_ap = bass.AP(tensor=bv2.tensor, offset=bv2.offset,
                            ap=[bv2.ap[0], [0, nb], [1, OUT]])
            nc.vector.tensor_tensor(out=rda, in0=rda, in1=b2_ap,
                                    op=mybir.AluOpType.add)
            for j, b in enumerate(blocks):
                nc.sync.dma_start(out_ext[b * 128:(b + 1) * 128, :],
                                  rd[:, j * OUT:(j + 1) * OUT])

    # ---------------- phase sequence ----------------
    with tc.tile_pool(name="featp", bufs=1) as featp:
        featT_sb = featp.tile([F, ROWS], f32)
        nc.sync.dma_start(featT_sb[:], featT[:])
        proj_phase(lambda nt: featT_sb[:, nt * 128:(nt + 1) * 128],
                   w1_sb, slice1, er1_sb)
        tc.strict_bb_all_engine_barrier()
        nc.gpsimd.collective_compute(
            "AllGather", mybir.AluOpType.bypass, replica_groups=[core_ids],
            ins=[slice1[:]], outs=[table1[:]])
        tc.strict_bb_all_engine_barrier()
        edge_phase(table1, er1_sb, 1)

    tc.strict_bb_all_engine_barrier()
    with tc.tile_pool(name="h1p", bufs=1) as h1p:
        h1T_sb = h1p.tile([128, ROWS], bf16)
        nc.sync.dma_start(h1T_sb[:], h1r[:], transpose=True)
        proj_phase(lambda nt: h1T_sb[:, nt * 128:(nt + 1) * 128],
                   w2_sb, slice2, er2_sb)
        tc.strict_bb_all_engine_barrier()
        nc.gpsimd.collective_compute(
            "AllGather", mybir.AluOpType.bypass, replica_groups=[core_ids],
            ins=[slice2[:]], outs=[table2[:]])
        tc.strict_bb_all_engine_barrier()
        edge_phase(table2, er2_sb, 2)


def build_nc(runs, T, compile=True):
    from concourse import bacc

    supmax = max(len(r["tiles"]) for r in runs)
    nc = bacc.Bacc("TRN2", target_bir_lowering=False, num_swdge_queues=4)
    with tile.TileContext(nc) as tc:
        build_kernel(tc, runs, T, supmax)
    if compile:
        nc.compile()
    return nc


def make_in_maps(per_core, feat, perms, W1, al1, ar1, b1, W2, al2, ar2, b2):
    w1a = aug_weights(np.asarray(W1, np.float32), np.asarray(al1, np.float32),
                      np.asarray(ar1, np.float32))
    w2a = aug_weights(np.asarray(W2, np.float32), np.asarray(al2, np.float32),
                      np.asarray(ar2, np.float32)).astype(ml_dtypes.bfloat16)
    iota = np.broadcast_to(np.arange(128, dtype=np.float32), (128, 128))
    iota = np.ascontiguousarray(iota.astype(ml_dtypes.bfloat16))
    sentel = np.full((1, 4), SENT_EL, np.float32).view(np.uint16).reshape(1, 8)
    sentel = sentel.view(ml_dtypes.bfloat16)
    b1r = np.ascontiguousarray(np.broadcast_to(
        np.asarray(b1, np.float32).reshape(1, F), (128, F)))
    b2m = np.asarray(b2, np.float32).reshape(H, OUT).mean(axis=0)
    b2r = np.ascontiguousarray(np.broadcast_to(b2m.reshape(1, OUT), (128, OUT)))
    feat = np.asarray(feat, np.float32)
    in_maps = []
    for c in range(NC):
        fs = np.zeros((ROWS, F), np.float32)
        fs[perms[c]] = feat[c * NPC:(c + 1) * NPC]
        m = dict(
            featT=np.ascontiguousarray(fs.T),
            w1aug=w1a, w2aug=w2a,
            osw=per_core[c]["osw"],
            dloc=per_core[c]["dloc"],
            iota=iota, b1rep=b1r, b2rep=b2r, sentel=sentel,
        )
        in_maps.append(m)
    return in_maps


_CACHE = {}


def _get_program(src, dst):
    per_core, perms, tb, runs, T = prep_inputs(src, dst)
    key = (T, tb.tobytes())
    if key not in _CACHE:
        _CACHE[key] = build_nc(runs, T)
    return _CACHE[key], per_core, perms


def kernel(feat, src, dst, W1, al1, ar1, b1, W2, al2, ar2, b2,
           _trace=False, _return_results=False):
    from concourse.bass_utils import run_bass_kernel_spmd

    nc, per_core, perms = _get_program(src, dst)
    in_maps = make_in_maps(per_core, feat, perms, W1, al1, ar1, b1,
                           W2, al2, ar2, b2)
    res = run_bass_kernel_spmd(nc, in_maps, list(range(NC)), trace=_trace)
    out = np.zeros((NC * NPC, OUT), np.float32)
    for c in range(NC):
        oc = np.asarray(res.results[c]["out"])
        out[c * NPC:(c + 1) * NPC] = oc[perms[c]]
    if _return_results:
        return out, res
    return out
